# revision 8
# baseline (speedup 1.0000x reference)
"""Trainium2 Bass kernel for: x + s -> LayerNorm(W) -> 2x2x2 avgpool -> exact GELU.

Input  x: (32, 32, 16, 32, 64) f32, sum_weight (1,), gamma (64,), beta (64,)
Output:   (32, 32, 8, 16, 32) f32

Math notes:
  v = x + s;  LN over last dim W: mean/var are shift-equivariant/invariant, so
  (v - mean_v) = (x - mean_x) and var_v = var_x  ==> sum_weight cancels exactly.
  ln = (x - mu) * rho * gamma + beta,  rho = rsqrt(var + eps)
  8*pooled[q, w'] = S - gw[w'] * (64*M4)/64 + 4*(beta_e+beta_o)[w']
    S   = sum_{r in quad} rho_r * (ga*x[r,2w'] + go*x[r,2w'+1])
    M4  = sum_{r in quad} mu_r * rho_r
  out = Gelu(8*pooled / 8)

Performance design (v2):
  * x is converted to bf16 on the host: halves HBM traffic and enables the
    DVE high-performance modes.
  * All large elementwise ops are emitted as InstTensorScalarPtr
    (scalar_tensor_tensor): 2x cycle rate with any SBUF operands, 4x with
    all-bf16 packed operands.  Plain tensor_tensor/tensor_reduce run at 1x.
  * Row sums (sum x, sum x^2 per 64-wide LN row) are computed with 6-level
    binary fold trees of STT adds at 4x, batched 4 chunks per instruction,
    instead of 1x tensor_reduce (which dominated the old kernel).
  * Stats postprocessing (var, rstd, mu*rho, corrections) is batched across
    all 8 chunks into a handful of [128, 512] ops.
  * Square work is split DVE/ACT; x*rstd is split DVE/GPSIMD (the rstd
    broadcast has a stride-0 free dim, capping DVE at 2x, so GPSIMD helps).
  * Output is written bf16 and upcast on the host.

Layout: data-parallel over batch N (4 per core x 8 cores). On each core,
partition dim = the 128 (n, c) pairs; free dim = (d, h, w).
"""

import numpy as np

import concourse.bacc as bacc
import concourse.bass as bass
import concourse.tile as tile
from concourse import mybir
from concourse.bass_utils import run_bass_kernel_spmd

P = 128
N, C, D, H, W = 32, 32, 16, 32, 64
NCORES = 8
NPER = N // NCORES  # batches per core
EPS = 1e-5
F32 = mybir.dt.float32
BF16 = mybir.dt.bfloat16
MULT = mybir.AluOpType.mult
ADD = mybir.AluOpType.add
SUB = mybir.AluOpType.subtract

CHUNK = 2 * H * W  # 4096 elems per partition (one d-pair)
NCHUNK = D // 2  # 8
ROWS = 2 * H  # 64 LN rows per chunk
GRP = 2  # chunks per tree group (keeps tree scratch within SBUF)
NGRP = NCHUNK // GRP

# knobs
SQ_ACT_CHUNKS = (0, 2, 4, 6)  # chunks whose x^2 is computed on ACT (rest DVE)
XR_GP_ROWS = 20  # rows (of 64) of x*rstd computed on GPSIMD


def _kernel_body(ctx, tc: tile.TileContext, out_ap, xs, cons):
    nc = tc.nc
    stt_v = nc.vector.scalar_tensor_tensor

    singles = ctx.enter_context(tc.tile_pool(name="singles", bufs=1))
    sqpool = ctx.enter_context(tc.tile_pool(name="sqpool", bufs=1))
    treep = ctx.enter_context(tc.tile_pool(name="treep", bufs=2))
    statp = ctx.enter_context(tc.tile_pool(name="statp", bufs=1))
    workp = ctx.enter_context(tc.tile_pool(name="workp", bufs=2))
    outp = ctx.enter_context(tc.tile_pool(name="outp", bufs=3))

    # ---- constants (bf16, broadcast to all partitions via DMA) ----
    gam_t = singles.tile([P, 64], BF16)  # full gamma
    gw_t = singles.tile([P, 32], BF16)  # (ga+go)/64
    bw_t = singles.tile([P, 32], BF16)  # 4*(beta_e+beta_o)
    nc.sync.dma_start(out=gam_t[:], in_=cons[0:1, :].to_broadcast((P, 64)))
    nc.sync.dma_start(out=gw_t[:], in_=cons[1:2, 0:32].to_broadcast((P, 32)))
    nc.sync.dma_start(out=bw_t[:], in_=cons[2:3, 0:32].to_broadcast((P, 32)))
    eps_t = singles.tile([P, 1], F32)
    nc.vector.memset(eps_t[:], EPS)

    # ---- resident x (all 8 chunks, bf16: 64KB/partition) ----
    x_all = singles.tile([P, NCHUNK, CHUNK], BF16)
    xsf = xs.rearrange("p d h w -> p (d h w)")
    for k in range(NCHUNK):
        nc.sync.dma_start(out=x_all[:, k], in_=xsf[:, k * CHUNK : (k + 1) * CHUNK])

    # stats accumulators, f32
    r1 = statp.tile([P, NCHUNK, ROWS], F32, tag="r1")  # sum x per row
    r2 = statp.tile([P, NCHUNK, ROWS], F32, tag="r2")  # sum x^2 per row

    # =================== Phase A: row sums via fold trees ===================
    # All APs kept <= 3D (walrus requires 2/3D for TensorScalarPtr).
    NR = GRP * ROWS  # rows per tree group

    def fold_tree(src3, dst2):
        # src3: [P, NR, 64] bf16 view; dst2: [P, NR] f32 view
        w = 32
        cur = src3
        while w >= 2:
            nxt_dtype = BF16 if w > 2 else F32
            nxt = treep.tile([P, NR, w], nxt_dtype, tag=f"tree{w}")
            stt_v(
                out=nxt[:],
                in0=cur[:, :, 0:w],
                scalar=1.0,
                in1=cur[:, :, w : 2 * w],
                op0=MULT,
                op1=ADD,
            )
            cur = nxt[:]
            w //= 2
        stt_v(
            out=dst2,
            in0=cur[:, :, 0],
            scalar=1.0,
            in1=cur[:, :, 1],
            op0=MULT,
            op1=ADD,
        )

    for g in range(NGRP):
        sq4 = sqpool.tile([P, GRP, CHUNK], BF16, tag="sq4")
        for j in range(GRP):
            k = GRP * g + j
            if k in SQ_ACT_CHUNKS:
                nc.scalar.activation(
                    sq4[:, j],
                    x_all[:, k],
                    mybir.ActivationFunctionType.Square,
                )
            else:
                stt_v(
                    out=sq4[:, j],
                    in0=x_all[:, k],
                    scalar=1.0,
                    in1=x_all[:, k],
                    op0=MULT,
                    op1=MULT,
                )
        x4 = x_all[:, GRP * g : GRP * (g + 1)].rearrange(
            "p g (r w) -> p (g r) w", w=W
        )
        fold_tree(
            x4,
            r1[:, GRP * g : GRP * (g + 1)].rearrange("p g r -> p (g r)"),
        )
        fold_tree(
            sq4[:].rearrange("p g (r w) -> p (g r) w", w=W),
            r2[:, GRP * g : GRP * (g + 1)].rearrange("p g r -> p (g r)"),
        )

    # =================== Phase B: batched stats ===================
    # 4096*var = 64*r2 - r1^2 ; rstd = 1/sqrt(4096*var/4096 + eps)
    NSTAT = NCHUNK * ROWS  # 512
    r1f = r1[:].rearrange("p k r -> p (k r)")
    r2f = r2[:].rearrange("p k r -> p (k r)")
    s1sq = statp.tile([P, NSTAT], F32, tag="s1sq")
    stt_v(out=s1sq[:], in0=r1f, scalar=1.0, in1=r1f, op0=MULT, op1=MULT)
    t64 = statp.tile([P, NSTAT], F32, tag="t64")
    stt_v(out=t64[:], in0=r2f, scalar=float(W), in1=s1sq[:], op0=MULT, op1=SUB)
    rstd = statp.tile([P, NSTAT], F32, tag="rstd")
    nc.scalar.activation(
        rstd[:],
        t64[:],
        mybir.ActivationFunctionType.Sqrt,
        bias=eps_t[:],
        scale=1.0 / (W * W),
    )
    nc.vector.reciprocal(out=rstd[:], in_=rstd[:])
    mrs = statp.tile([P, NSTAT], F32, tag="mrs")  # 64*mu*rho
    stt_v(out=mrs[:], in0=r1f, scalar=1.0, in1=rstd[:], op0=MULT, op1=MULT)

    # quad-pool mrs -> mq [P, NCHUNK, 16]; z = bw - gw*mq  (bf16)
    mrs4 = mrs[:].rearrange("p (k dd h) -> p k dd h", k=NCHUNK, dd=2)
    m1 = statp.tile([P, NCHUNK, H], F32, tag="m1")
    stt_v(
        out=m1[:], in0=mrs4[:, :, 0], scalar=1.0, in1=mrs4[:, :, 1],
        op0=MULT, op1=ADD,
    )
    m1p = m1[:].rearrange("p k (h t) -> p k h t", t=2)
    mq = statp.tile([P, NCHUNK, H // 2], BF16, tag="mq")
    stt_v(
        out=mq[:], in0=m1p[:, :, :, 0], scalar=1.0, in1=m1p[:, :, :, 1],
        op0=MULT, op1=ADD,
    )
    # z_all[k, h', w'] = bw[w'] - gw[w']*mq[k, h']   (flattened q = (k, h'))
    NQ = NCHUNK * (H // 2)  # 128
    zneg = statp.tile([P, NQ, 32], BF16, tag="zneg")
    mqf = mq[:].rearrange("p k h -> p (k h)")
    stt_v(
        out=zneg[:],
        in0=mqf.unsqueeze(2).to_broadcast((P, NQ, 32)),
        scalar=-1.0,
        in1=gw_t[:].unsqueeze(1).to_broadcast((P, NQ, 32)),
        op0=MULT,
        op1=MULT,
    )
    z_all = statp.tile([P, NQ, 32], BF16, tag="z_all")
    stt_v(
        out=z_all[:],
        in0=zneg[:],
        scalar=1.0,
        in1=bw_t[:].unsqueeze(1).to_broadcast((P, NQ, 32)),
        op0=MULT,
        op1=ADD,
    )

    # =================== Phase C: normalize + pool + gelu ===================
    rstd3 = rstd[:].rearrange("p (k r) -> p k r", k=NCHUNK)
    outf = out_ap.rearrange("p d h w -> p d (h w)")
    for k in range(NCHUNK):
        xc3 = x_all[:, k].rearrange("p (r w) -> p r w", w=W)
        xr = workp.tile([P, ROWS, W], BF16, tag="xr")
        g = XR_GP_ROWS
        if g > 0:
            nc.gpsimd.tensor_tensor(
                out=xr[:, :g],
                in0=xc3[:, :g],
                in1=rstd3[:, k, :g].unsqueeze(2).to_broadcast((P, g, W)),
                op=MULT,
            )
        stt_v(
            out=xr[:, g:],
            in0=xc3[:, g:],
            scalar=1.0,
            in1=rstd3[:, k, g:].unsqueeze(2).to_broadcast((P, ROWS - g, W)),
            op0=MULT,
            op1=MULT,
        )
        # d-pool: [P, 2, H*W] -> [P, H*W]
        xrd = xr[:].rearrange("p (dd h) w -> p dd (h w)", dd=2)
        xd = workp.tile([P, H * W], BF16, tag="xd")
        stt_v(
            out=xd[:], in0=xrd[:, 0], scalar=1.0, in1=xrd[:, 1],
            op0=MULT, op1=ADD,
        )
        # h-pool: [P, 16, 2, W] -> [P, 16, W]
        xd4 = xd[:].rearrange("p (h t w) -> p h t w", t=2, w=W)
        xh = workp.tile([P, H // 2, W], BF16, tag="xh")
        stt_v(
            out=xh[:], in0=xd4[:, :, 0], scalar=1.0, in1=xd4[:, :, 1],
            op0=MULT, op1=ADD,
        )
        # gamma: xg = xh * gamma (full, broadcast over h')
        xg = workp.tile([P, H // 2, W], BF16, tag="xg")
        stt_v(
            out=xg[:],
            in0=xh[:],
            scalar=1.0,
            in1=gam_t[:].unsqueeze(1).to_broadcast((P, H // 2, W)),
            op0=MULT,
            op1=MULT,
        )
        # w-pair: s = xg_e + xg_o -> [P, 16, 32]
        xg4 = xg[:].rearrange("p h (v t) -> p h v t", t=2)
        spre = workp.tile([P, H // 2, 32], BF16, tag="spre")
        stt_v(
            out=spre[:], in0=xg4[:, :, :, 0], scalar=1.0, in1=xg4[:, :, :, 1],
            op0=MULT, op1=ADD,
        )
        # + correction
        pre = workp.tile([P, H // 2, 32], BF16, tag="pre")
        stt_v(
            out=pre[:],
            in0=spre[:],
            scalar=1.0,
            in1=z_all[:, k * (H // 2) : (k + 1) * (H // 2)],
            op0=MULT,
            op1=ADD,
        )
        res = outp.tile([P, (H // 2) * 32], BF16, tag="res")
        nc.scalar.activation(
            res[:],
            pre[:].rearrange("p a b -> p (a b)"),
            mybir.ActivationFunctionType.Gelu,
            scale=0.125,
        )
        nc.sync.dma_start(out=outf[:, k], in_=res[:])


_CACHE: dict = {}


def _get_compiled():
    if "nc" not in _CACHE:
        nc = bacc.Bacc("TRN2", target_bir_lowering=False, debug=False)
        xs = nc.dram_tensor("xs", [P, D, H, W], BF16, kind="ExternalInput").ap()
        cons = nc.dram_tensor("cons", [3, 64], BF16, kind="ExternalInput").ap()
        out = nc.dram_tensor(
            "out", [P, D // 2, H // 2, W // 2], BF16, kind="ExternalOutput"
        ).ap()
        from contextlib import ExitStack

        with tile.TileContext(nc) as tc, ExitStack() as ctx:
            _kernel_body(ctx, tc, out, xs, cons)
        nc.compile()
        _CACHE["nc"] = nc
    return _CACHE["nc"]


def _make_cons(gamma: np.ndarray, beta: np.ndarray) -> np.ndarray:
    import ml_dtypes

    ga = gamma[0::2].astype(np.float32)
    go = gamma[1::2].astype(np.float32)
    cons = np.zeros((3, 64), np.float32)
    cons[0] = gamma
    cons[1, 0:32] = (ga + go) / float(W)  # mrs carries an extra factor of 64
    cons[2, 0:32] = 4.0 * (beta[0::2] + beta[1::2])
    return cons.astype(ml_dtypes.bfloat16)


def kernel(x, sum_weight, gamma, beta, trace=False):
    import ml_dtypes

    del sum_weight  # cancels exactly in LayerNorm (shift invariance)
    nc = _get_compiled()
    x = np.asarray(x)
    if x.dtype != ml_dtypes.bfloat16:
        x = x.astype(ml_dtypes.bfloat16)
    x = np.ascontiguousarray(x)
    cons = _make_cons(
        np.asarray(gamma, dtype=np.float32), np.asarray(beta, dtype=np.float32)
    )
    in_maps = []
    for core in range(NCORES):
        shard = x[core * NPER : (core + 1) * NPER].reshape(P, D, H, W)
        in_maps.append({"xs": shard, "cons": cons})
    res = run_bass_kernel_spmd(nc, in_maps, core_ids=list(range(NCORES)), trace=trace)
    out = np.concatenate(
        [
            res.results[i]["out"]
            .astype(np.float32)
            .reshape(NPER, C, D // 2, H // 2, W // 2)
            for i in range(NCORES)
        ],
        axis=0,
    )
    if trace:
        return out, res
    return out


if __name__ == "__main__":
    rng = np.random.default_rng(0)
    x = rng.standard_normal((N, C, D, H, W), dtype=np.float32)
    sw = rng.standard_normal((1,)).astype(np.float32)
    gamma = rng.random((W,), dtype=np.float32)
    beta = rng.standard_normal((W,)).astype(np.float32)
    y = kernel(x, sw, gamma, beta)
    print(y.shape, y.dtype)


# revision 19
# speedup vs baseline: 1.1169x; 1.1169x over previous
"""Trainium2 Bass kernel for: x + s -> LayerNorm(W) -> 2x2x2 avgpool -> exact GELU.

Input  x: (32, 32, 16, 32, 64) f32, sum_weight (1,), gamma (64,), beta (64,)
Output:   (32, 32, 8, 16, 32) f32

Math:
  sum_weight cancels exactly (LN shift invariance).
  ln = (x - mu) * rho * gamma + beta,  rho = rsqrt(var + eps)
  8*pooled[q=(k,h'), w'] = S - (ga+go)[w']*M4 + 4*(be+bo)[w']
    S  = sum_{dd,hp,w in pair} gamma[w] * rho_row * x[row, w]
    M4 = sum_{quad rows} mu*rho
  out = Gelu(8*pooled / 8)

Performance design (v3) — measured engine rates are ~1 elem/ns/partition on
DVE/ACT/GPSIMD with no fast modes, so the only spare capacity is the Tensor
engine.  Layout: data-parallel over N; per core partitions = 128 (n,c) pairs;
chunks = one d-pair, host-permuted to (h, dd, w) order so a DMA transpose
yields partitions (dd, w).

  Phase A (per chunk): DMA in bf16 -> DMA-transpose xT[(dd,w), (h,nc)] ->
    ACT square -> PE row-sum matmuls for sum(x), sum(x^2) (contraction over
    w per d-parity, quarter-sized PSUM tiles) -> drain to [nc, (k,dd,h)].
  Phase B (once): batched var/rstd/mu*rho/quad-sum math on [128, 512] tiles;
    mq transposed via xbar for the PE correction rows.
  Phase C (per chunk): xr = x*rstd (DVE, the one remaining elementwise
    pass) -> DMA-transpose -> ONE accumulating PE pass does d-pool (lhsT
    sums dd), gamma*w-pair combine (lhsT weights), h-pool (PSUM accumulate
    over h-parity matmuls) and the -gw*M4 correction (K=1 matmul) -> ACT
    Gelu straight off PSUM with per-partition beta bias -> DMA out.

  ACT uses only {square, gelu} (same table set) + one batched sqrt.
"""

import numpy as np

import concourse.bacc as bacc
import concourse.bass as bass
import concourse.tile as tile
from concourse import mybir
from concourse.bass_utils import run_bass_kernel_spmd

P = 128
N, C, D, H, W = 32, 32, 16, 32, 64
NCORES = 8
NPER = N // NCORES
EPS = 1e-5
F32 = mybir.dt.float32
BF16 = mybir.dt.bfloat16
MULT = mybir.AluOpType.mult
ADD = mybir.AluOpType.add
SUB = mybir.AluOpType.subtract

NCHUNK = D // 2  # 8 chunks = d-pairs
CHUNK = 2 * H * W  # 4096 elems/partition: (h32, dd2, w64)
ROWS = 2 * H  # 64 LN rows per chunk, order (h, dd)
HH = H // 2  # 16
WW = W // 2  # 32


def _kernel_body(ctx, tc: tile.TileContext, out_ap, xs, cons, consG, bwT_d):
    nc = tc.nc
    stt = nc.vector.scalar_tensor_tensor
    ACTF = mybir.ActivationFunctionType

    singles = ctx.enter_context(tc.tile_pool(name="singles", bufs=1))
    statp = ctx.enter_context(tc.tile_pool(name="statp", bufs=1))
    xrp = ctx.enter_context(tc.tile_pool(name="xrp", bufs=2))
    xrtp = ctx.enter_context(tc.tile_pool(name="xrtp", bufs=2))
    outp = ctx.enter_context(tc.tile_pool(name="outp", bufs=2))

    # constants.  cons rows: r0/r1 = dd-parity indicators, r2 = gwneg [32 pad]
    sel_t = singles.tile([P, 2], BF16)
    nc.sync.dma_start(out=sel_t[:, 0:1], in_=cons[0:1, :].rearrange("a b -> b a"))
    nc.sync.dma_start(out=sel_t[:, 1:2], in_=cons[1:2, :].rearrange("a b -> b a"))
    gwneg = singles.tile([1, WW], BF16)
    nc.sync.dma_start(out=gwneg[:], in_=cons[2:3, 0:WW])
    G_t = singles.tile([P, WW], BF16)  # gamma-pair lhsT
    nc.sync.dma_start(out=G_t[:], in_=consG[:, :])
    bwT = singles.tile([WW, 1], F32)
    nc.sync.dma_start(out=bwT[:], in_=bwT_d[:, :])
    eps_t = singles.tile([P, 1], F32)
    nc.vector.memset(eps_t[:], EPS)

    x_all = singles.tile([P, NCHUNK, CHUNK], BF16)  # 64KB/partition
    xsf = xs.rearrange("p k f -> p (k f)")

    # ====== Phase A: load; r1 via PE (x-block stationary); r2 via reduce ======
    # r1: per h-block t, lhsT = xT[:, t, :] [128(dd,w), 128nc], rhs = sel ->
    #     psum[nc, t, dd].  r2: ACT square (f32) + GPSIMD/DVE row-reduce.
    r1sb = statp.tile([P, NCHUNK, H, 2], F32, tag="r1")  # (k, h, dd)
    r2sb = statp.tile([P, NCHUNK, H, 2], F32, tag="r2")
    xtp = ctx.enter_context(tc.tile_pool(name="xtp", bufs=2))
    sqp = ctx.enter_context(tc.tile_pool(name="sqp", bufs=2))
    R2_GP_CHUNKS = (0, 1, 2, 4, 5, 6)  # r2 reduce on GPSIMD for these
    with tc.tile_pool(name="psR", space="PSUM", bufs=2) as psR:
        for k in range(NCHUNK):
            nc.sync.dma_start(
                out=x_all[:, k], in_=xsf[:, k * CHUNK : (k + 1) * CHUNK]
            )
            xT = xtp.tile([P, H, P], BF16, tag="xT")  # [(dd,w), h, nc]
            nc.sync.dma_start_transpose(out=xT[:], in_=x_all[:, k])
            pr = psR.tile([P, H, 2], F32, tag="pr")
            for t in range(H):
                nc.tensor.matmul(pr[:, t, :], lhsT=xT[:, t, :], rhs=sel_t[:],
                                 start=True, stop=True)
            nc.vector.tensor_copy(out=r1sb[:, k], in_=pr[:])
            sqf = sqp.tile([P, ROWS, W], F32, tag="sqf")
            nc.scalar.activation(
                sqf[:].rearrange("p r w -> p (r w)"), x_all[:, k], ACTF.Square
            )
            nc.vector.tensor_reduce(
                out=r2sb[:, k].rearrange("p h dd -> p (h dd)"),
                in_=sqf[:],
                axis=mybir.AxisListType.X,
                op=ADD,
            )

    # =================== Phase B: batched stats extraction ===================
    # 4096*var = 64*r2 - r1^2 ; rstd = rsqrt(var + eps) ; mrs = 64*mu*rho
    NS = NCHUNK * ROWS  # 512
    r1f = r1sb[:].rearrange("p k h dd -> p (k h dd)")
    r2f = r2sb[:].rearrange("p k h dd -> p (k h dd)")
    s1sq = statp.tile([P, NS], F32, tag="s1sq")
    stt(out=s1sq[:], in0=r1f, scalar=1.0, in1=r1f, op0=MULT, op1=MULT)
    t64 = statp.tile([P, NS], F32, tag="t64")
    stt(out=t64[:], in0=r2f, scalar=float(W), in1=s1sq[:], op0=MULT, op1=SUB)
    rstd = statp.tile([P, NS], F32, tag="rstd")
    nc.scalar.activation(
        rstd[:], t64[:], ACTF.Sqrt, bias=eps_t[:], scale=1.0 / (W * W)
    )
    nc.vector.reciprocal(out=rstd[:], in_=rstd[:])
    # mrs = 64*mu*rho = r1*rstd ; rows already in (k, h, dd) order
    mrs = statp.tile([P, NS], F32, tag="mrs")
    stt(out=mrs[:], in0=r1f, scalar=1.0, in1=rstd[:], op0=MULT, op1=MULT)
    mrs4 = mrs[:].rearrange("p (k h dd) -> p k h dd", k=NCHUNK, dd=2)
    m1 = statp.tile([P, NCHUNK, H], F32, tag="m1")
    stt(out=m1[:], in0=mrs4[:, :, :, 0], scalar=1.0, in1=mrs4[:, :, :, 1],
        op0=MULT, op1=ADD)
    m1p = m1[:].rearrange("p k (hh t) -> p k hh t", t=2)
    mq = statp.tile([P, NCHUNK, HH], BF16, tag="mq")
    stt(out=mq[:], in0=m1p[:, :, :, 0], scalar=1.0, in1=m1p[:, :, :, 1],
        op0=MULT, op1=ADD)
    # transpose mq [nc, (k,h')] -> mqT [(k,h'), nc]; then per-chunk rows
    mqT = statp.tile([P, 1, P], BF16, tag="mqT")
    nc.sync.dma_start_transpose(
        out=mqT[:], in_=mq[:].rearrange("p k h -> p (k h)")
    )
    mqrow = statp.tile([1, NCHUNK, HH, P], BF16, tag="mqrow")
    for k in range(NCHUNK):
        nc.sync.dma_start(
            out=mqrow[:, k], in_=mqT[k * HH : (k + 1) * HH, 0, :]
        )

    # ============ Phase C: normalize, transpose, PE pool pass ============
    outf = out_ap.rearrange("w k h n -> w k (h n)")
    with tc.tile_pool(name="psC", space="PSUM", bufs=2) as psC:
        for k in range(NCHUNK):
            xc = x_all[:, k].rearrange("p (r w) -> p r w", w=W)
            xr = xrp.tile([P, ROWS, W], BF16, tag="xr")
            stt(
                out=xr[:],
                in0=xc,
                scalar=1.0,
                in1=rstd[:, k * ROWS : (k + 1) * ROWS]
                .unsqueeze(2)
                .to_broadcast((P, ROWS, W)),
                op0=MULT,
                op1=MULT,
            )
            xrT = xrtp.tile([P, H, P], BF16, tag="xrT")  # [(dd,w), h, nc]
            nc.sync.dma_start_transpose(
                out=xrT[:], in_=xr[:].rearrange("p r w -> p (r w)")
            )
            pc = psC.tile([WW, HH, P], F32, tag="pc")
            xrT4 = xrT[:].rearrange("p (hh t) n -> p hh t n", t=2)
            # moving-operand limit is 512 elems: 4 h'-slices of 4x128 each
            for s in range(4):
                hh = slice(4 * s, 4 * (s + 1))
                nc.tensor.matmul(pc[:, hh, :], lhsT=G_t[:],
                                 rhs=xrT4[:, hh, 0, :],
                                 start=True, stop=False)
                nc.tensor.matmul(pc[:, hh, :], lhsT=G_t[:],
                                 rhs=xrT4[:, hh, 1, :],
                                 start=False, stop=False)
                nc.tensor.matmul(pc[:, hh, :], lhsT=gwneg[:],
                                 rhs=mqrow[:, k, hh, :],
                                 start=False, stop=True)
            res = outp.tile([WW, HH * P], BF16, tag="res")
            nc.scalar.activation(
                res[:],
                pc[:].rearrange("w h n -> w (h n)"),
                ACTF.Gelu,
                scale=0.125,
                bias=bwT[:],
            )
            nc.sync.dma_start(out=outf[:, k], in_=res[:])


_CACHE: dict = {}


def _get_compiled():
    if "nc" not in _CACHE:
        nc = bacc.Bacc("TRN2", target_bir_lowering=False, debug=False)
        xs = nc.dram_tensor("xs", [P, NCHUNK, CHUNK], BF16, kind="ExternalInput").ap()
        cons = nc.dram_tensor("cons", [3, P], BF16, kind="ExternalInput").ap()
        consG = nc.dram_tensor("consG", [P, WW], BF16, kind="ExternalInput").ap()
        bwT_d = nc.dram_tensor("bwT", [WW, 1], F32, kind="ExternalInput").ap()
        out = nc.dram_tensor(
            "out", [WW, NCHUNK, HH, P], BF16, kind="ExternalOutput"
        ).ap()
        from contextlib import ExitStack

        with tile.TileContext(nc) as tc, ExitStack() as ctx:
            _kernel_body(ctx, tc, out, xs, cons, consG, bwT_d)
        nc.compile()
        _CACHE["nc"] = nc
    return _CACHE["nc"]


def _make_consts(gamma: np.ndarray, beta: np.ndarray):
    import ml_dtypes

    # cons rows (bf16, width 128): sel_dd0, sel_dd1, gwneg(pad)
    cons = np.zeros((3, P), np.float32)
    dd_of_p = (np.arange(P) // W).astype(np.int32)  # partition = dd*64 + w
    cons[0] = (dd_of_p == 0).astype(np.float32)
    cons[1] = (dd_of_p == 1).astype(np.float32)
    gw = gamma[0::2] + gamma[1::2]
    cons[2, 0:WW] = -gw / float(W)
    # G [128, 32]: G[(dd,w), w'] = gamma[w] if w in {2w', 2w'+1}
    G = np.zeros((P, WW), np.float32)
    for p in range(P):
        w = p % W
        G[p, w // 2] = gamma[w]
    bwT = ((beta[0::2] + beta[1::2]) / 2.0).astype(np.float32).reshape(WW, 1)
    return (
        cons.astype(ml_dtypes.bfloat16),
        np.ascontiguousarray(G.astype(ml_dtypes.bfloat16)),
        bwT,
    )


def kernel(x, sum_weight, gamma, beta, trace=False):
    import ml_dtypes

    del sum_weight  # cancels exactly in LayerNorm (shift invariance)
    nc = _get_compiled()
    x = np.asarray(x)
    if x.dtype != ml_dtypes.bfloat16:
        x = x.astype(ml_dtypes.bfloat16)
    # (N, C, D, H, W) -> (N, C, k, h, dd, w): chunk free order (h, dd, w)
    xp = np.ascontiguousarray(
        x.reshape(N, C, NCHUNK, 2, H, W).transpose(0, 1, 2, 4, 3, 5)
    )
    cons, G, bwT = _make_consts(
        np.asarray(gamma, dtype=np.float32), np.asarray(beta, dtype=np.float32)
    )
    in_maps = []
    for core in range(NCORES):
        shard = xp[core * NPER : (core + 1) * NPER].reshape(P, NCHUNK, CHUNK)
        in_maps.append({"xs": shard, "cons": cons, "consG": G, "bwT": bwT})
    res = run_bass_kernel_spmd(nc, in_maps, core_ids=list(range(NCORES)), trace=trace)
    outs = []
    for i in range(NCORES):
        o = res.results[i]["out"].astype(np.float32)  # [w', k, h', nc]
        o = o.transpose(3, 1, 2, 0).reshape(NPER, C, NCHUNK, HH, WW)
        outs.append(o)
    out = np.concatenate(outs, axis=0)
    if trace:
        return out, res
    return out


if __name__ == "__main__":
    rng = np.random.default_rng(0)
    x = rng.standard_normal((N, C, D, H, W), dtype=np.float32)
    sw = rng.standard_normal((1,)).astype(np.float32)
    gamma = rng.random((W,), dtype=np.float32)
    beta = rng.standard_normal((W,)).astype(np.float32)
    y = kernel(x, sw, gamma, beta)
    print(y.shape, y.dtype)


# revision 20
# speedup vs baseline: 1.1821x; 1.0584x over previous
"""Trainium2 Bass kernel for: x + s -> LayerNorm(W) -> 2x2x2 avgpool -> exact GELU.

Input  x: (32, 32, 16, 32, 64) f32, sum_weight (1,), gamma (64,), beta (64,)
Output:   (32, 32, 8, 16, 32) f32

Math:
  sum_weight cancels exactly (LN shift invariance).
  ln = (x - mu) * rho * gamma + beta,  rho = rsqrt(var + eps)
  8*pooled[q, w'] = S - (ga+go)[w']*M4 + 4*(be+bo)[w'] ; out = Gelu(pooled)

Performance design (v4), based on measured TRN2 rates (all vector-ish engines
run ~1 elem/ns/partition, no fast modes; DMA-transpose is descriptor-bound
and unusable at volume; PE matmuls cost ~30-80 ns each):

  * x is sent twice from the host: normal layout [nc, (k,h,dd,w)] and
    pre-transposed [(dd,w), (k,h,nc)].  The extra 8.4 MB HBM read replaces
    ~100us of on-device xbar transposes.
  * Row sums (sum x, sum x^2 over W per d-parity) run on the otherwise-idle
    PE: per h-block, stationary = xT/sqT block [128, 128nc], moving = the
    2-column dd-parity selector -> psum [nc, h, dd].  Cheap 1-bank PSUM,
    drained by a 64-elem DVE copy.
  * x^2 on ACT (square shares a table set with gelu - no table thrash).
  * DVE keeps only: xr = x*rstd (f32 out), h-pool, gamma stage, w-pair,
    +correction, batched stats math.  GPSIMD takes the f32 d-pool.
  * Correction z = bw - gw*quadsum(mu*rho) precomputed once, batched.

Layout: data-parallel over batch N (4 per core x 8 cores); partitions = 128
(n,c); chunk = one d-pair in (h, dd, w) order.
"""

import numpy as np

import concourse.bacc as bacc
import concourse.bass as bass
import concourse.tile as tile
from concourse import mybir
from concourse.bass_utils import run_bass_kernel_spmd

P = 128
N, C, D, H, W = 32, 32, 16, 32, 64
NCORES = 8
NPER = N // NCORES
EPS = 1e-5
F32 = mybir.dt.float32
BF16 = mybir.dt.bfloat16
MULT = mybir.AluOpType.mult
ADD = mybir.AluOpType.add
SUB = mybir.AluOpType.subtract

NCHUNK = D // 2  # 8
CHUNK = 2 * H * W  # 4096: (h32, dd2, w64)
ROWS = 2 * H  # 64 rows per chunk, (h, dd) order
HH = H // 2  # 16
WW = W // 2  # 32


def _kernel_body(ctx, tc: tile.TileContext, out_ap, xs, xsT, cons):
    nc = tc.nc
    stt = nc.vector.scalar_tensor_tensor
    ACTF = mybir.ActivationFunctionType

    singles = ctx.enter_context(tc.tile_pool(name="singles", bufs=1))
    statp = ctx.enter_context(tc.tile_pool(name="statp", bufs=1))
    xtp = ctx.enter_context(tc.tile_pool(name="xtp", bufs=2))
    sqp = ctx.enter_context(tc.tile_pool(name="sqp", bufs=2))
    xrp = ctx.enter_context(tc.tile_pool(name="xrp", bufs=2))
    workp = ctx.enter_context(tc.tile_pool(name="workp", bufs=2))
    outp = ctx.enter_context(tc.tile_pool(name="outp", bufs=2))

    # cons rows (bf16 [5, 128]): sel0, sel1, gamma(64), gw(32), bw(32)
    sel_t = singles.tile([P, 2], BF16)
    nc.sync.dma_start(out=sel_t[:, 0:1], in_=cons[0:1, :].rearrange("a b -> b a"))
    nc.sync.dma_start(out=sel_t[:, 1:2], in_=cons[1:2, :].rearrange("a b -> b a"))
    gam_t = singles.tile([P, W], BF16)
    nc.sync.dma_start(out=gam_t[:], in_=cons[2:3, 0:W].to_broadcast((P, W)))
    gw_t = singles.tile([P, WW], BF16)
    nc.sync.dma_start(out=gw_t[:], in_=cons[3:4, 0:WW].to_broadcast((P, WW)))
    bw_t = singles.tile([P, WW], BF16)
    nc.sync.dma_start(out=bw_t[:], in_=cons[4:5, 0:WW].to_broadcast((P, WW)))
    eps_t = singles.tile([P, 1], F32)
    nc.vector.memset(eps_t[:], EPS)

    x_all = singles.tile([P, NCHUNK, CHUNK], BF16)  # 64KB/partition
    xsf = xs.rearrange("p k f -> p (k f)")
    r1sb = statp.tile([P, NCHUNK, H, 2], F32, tag="r1")  # (k, h, dd)
    r2sb = statp.tile([P, NCHUNK, H, 2], F32, tag="r2")

    # ============ Phase A: load; PE row-sums of x and x^2 ============
    with tc.tile_pool(name="psR", space="PSUM", bufs=2) as psR, tc.tile_pool(
        name="psS", space="PSUM", bufs=2
    ) as psS:
        for k in range(NCHUNK):
            nc.sync.dma_start(
                out=x_all[:, k], in_=xsf[:, k * CHUNK : (k + 1) * CHUNK]
            )
            xT = xtp.tile([P, H, P], BF16, tag="xT")  # [(dd,w), h, nc]
            nc.sync.dma_start(out=xT[:], in_=xsT[:, k])
            sqT = sqp.tile([P, H, P], BF16, tag="sqT")
            nc.scalar.activation(
                sqT[:].rearrange("p h n -> p (h n)"),
                xT[:].rearrange("p h n -> p (h n)"),
                ACTF.Square,
            )
            pr = psR.tile([P, H, 2], F32, tag="pr")
            ps = psS.tile([P, H, 2], F32, tag="ps")
            for t in range(H):
                nc.tensor.matmul(pr[:, t, :], lhsT=xT[:, t, :], rhs=sel_t[:],
                                 start=True, stop=True)
            nc.vector.tensor_copy(out=r1sb[:, k], in_=pr[:])
            for t in range(H):
                nc.tensor.matmul(ps[:, t, :], lhsT=sqT[:, t, :], rhs=sel_t[:],
                                 start=True, stop=True)
            nc.vector.tensor_copy(out=r2sb[:, k], in_=ps[:])

    # =================== Phase B: batched stats ===================
    # 4096*var = 64*r2 - r1^2 ; rstd = rsqrt(var+eps) ; mrs = r1*rstd = 64*mu*rho
    NS = NCHUNK * ROWS  # 512
    r1f = r1sb[:].rearrange("p k h dd -> p (k h dd)")
    r2f = r2sb[:].rearrange("p k h dd -> p (k h dd)")
    s1sq = statp.tile([P, NS], F32, tag="s1sq")
    stt(out=s1sq[:], in0=r1f, scalar=1.0, in1=r1f, op0=MULT, op1=MULT)
    t64 = statp.tile([P, NS], F32, tag="t64")
    stt(out=t64[:], in0=r2f, scalar=float(W), in1=s1sq[:], op0=MULT, op1=SUB)
    rstd = statp.tile([P, NS], F32, tag="rstd")
    nc.scalar.activation(
        rstd[:], t64[:], ACTF.Sqrt, bias=eps_t[:], scale=1.0 / (W * W)
    )
    nc.vector.reciprocal(out=rstd[:], in_=rstd[:])
    mrs = statp.tile([P, NS], F32, tag="mrs")
    stt(out=mrs[:], in0=r1f, scalar=1.0, in1=rstd[:], op0=MULT, op1=MULT)
    mrs4 = mrs[:].rearrange("p (k h dd) -> p k h dd", k=NCHUNK, dd=2)
    m1 = statp.tile([P, NCHUNK, H], F32, tag="m1")
    stt(out=m1[:], in0=mrs4[:, :, :, 0], scalar=1.0, in1=mrs4[:, :, :, 1],
        op0=MULT, op1=ADD)
    m1p = m1[:].rearrange("p k (hh t) -> p k hh t", t=2)
    mq = statp.tile([P, NCHUNK, HH], F32, tag="mq")
    stt(out=mq[:], in0=m1p[:, :, :, 0], scalar=1.0, in1=m1p[:, :, :, 1],
        op0=MULT, op1=ADD)
    # z[q=(k,h'), w'] = bw - gw*mq/64  (gw scaled on host by 1/64)
    NQ = NCHUNK * HH  # 128
    mqf = mq[:].rearrange("p k h -> p (k h)")
    zneg = statp.tile([P, NQ, WW], BF16, tag="zneg")
    stt(out=zneg[:], in0=mqf.unsqueeze(2).to_broadcast((P, NQ, WW)),
        scalar=-1.0, in1=gw_t[:].unsqueeze(1).to_broadcast((P, NQ, WW)),
        op0=MULT, op1=MULT)
    z_all = statp.tile([P, NQ, WW], BF16, tag="z_all")
    stt(out=z_all[:], in0=zneg[:], scalar=1.0,
        in1=bw_t[:].unsqueeze(1).to_broadcast((P, NQ, WW)),
        op0=MULT, op1=ADD)

    # ============ Phase C: normalize + pool + gelu (elementwise) ============
    outf = out_ap.rearrange("p k f -> p k f")
    for k in range(NCHUNK):
        xc = x_all[:, k].rearrange("p (r w) -> p r w", w=W)
        xr = xrp.tile([P, ROWS, W], F32, tag="xr")
        stt(out=xr[:], in0=xc, scalar=1.0,
            in1=rstd[:, k * ROWS : (k + 1) * ROWS]
            .unsqueeze(2).to_broadcast((P, ROWS, W)),
            op0=MULT, op1=MULT)
        # d-pool on GPSIMD (f32): rows (h, dd) -> sum over dd
        xr4 = xr[:].rearrange("p (h dd) w -> p h dd w", dd=2)
        xd = workp.tile([P, H, W], F32, tag="xd")
        nc.gpsimd.tensor_tensor(out=xd[:], in0=xr4[:, :, 0, :],
                                in1=xr4[:, :, 1, :], op=ADD)
        # h-pool (DVE): [P, 16, 2, W] -> [P, 16, W]
        xd4 = xd[:].rearrange("p (hh t) w -> p hh t w", t=2)
        xh = workp.tile([P, HH, W], F32, tag="xh")
        stt(out=xh[:], in0=xd4[:, :, 0, :], scalar=1.0, in1=xd4[:, :, 1, :],
            op0=MULT, op1=ADD)
        # gamma
        xg = workp.tile([P, HH, W], F32, tag="xg")
        stt(out=xg[:], in0=xh[:], scalar=1.0,
            in1=gam_t[:].unsqueeze(1).to_broadcast((P, HH, W)),
            op0=MULT, op1=MULT)
        # w-pair
        xg4 = xg[:].rearrange("p h (v t) -> p h v t", t=2)
        spre = workp.tile([P, HH, WW], F32, tag="spre")
        stt(out=spre[:], in0=xg4[:, :, :, 0], scalar=1.0, in1=xg4[:, :, :, 1],
            op0=MULT, op1=ADD)
        # + correction
        pre = workp.tile([P, HH, WW], F32, tag="pre")
        stt(out=pre[:], in0=spre[:], scalar=1.0,
            in1=z_all[:, k * HH : (k + 1) * HH], op0=MULT, op1=ADD)
        res = outp.tile([P, HH * WW], BF16, tag="res")
        nc.scalar.activation(
            res[:], pre[:].rearrange("p a b -> p (a b)"), ACTF.Gelu, scale=0.125
        )
        nc.sync.dma_start(out=outf[:, k], in_=res[:])


_CACHE: dict = {}


def _get_compiled():
    if "nc" not in _CACHE:
        nc = bacc.Bacc("TRN2", target_bir_lowering=False, debug=False)
        xs = nc.dram_tensor("xs", [P, NCHUNK, CHUNK], BF16, kind="ExternalInput").ap()
        xsT = nc.dram_tensor(
            "xsT", [P, NCHUNK, H, P], BF16, kind="ExternalInput"
        ).ap()
        cons = nc.dram_tensor("cons", [5, P], BF16, kind="ExternalInput").ap()
        out = nc.dram_tensor(
            "out", [P, NCHUNK, HH * WW], BF16, kind="ExternalOutput"
        ).ap()
        from contextlib import ExitStack

        with tile.TileContext(nc) as tc, ExitStack() as ctx:
            _kernel_body(ctx, tc, out, xs, xsT, cons)
        nc.compile()
        _CACHE["nc"] = nc
    return _CACHE["nc"]


def _make_consts(gamma: np.ndarray, beta: np.ndarray):
    import ml_dtypes

    cons = np.zeros((5, P), np.float32)
    dd_of_p = (np.arange(P) // W).astype(np.int32)
    cons[0] = (dd_of_p == 0).astype(np.float32)
    cons[1] = (dd_of_p == 1).astype(np.float32)
    cons[2, 0:W] = gamma
    cons[3, 0:WW] = (gamma[0::2] + gamma[1::2]) / float(W)  # mrs carries 64x
    cons[4, 0:WW] = 4.0 * (beta[0::2] + beta[1::2])
    return cons.astype(ml_dtypes.bfloat16)


def kernel(x, sum_weight, gamma, beta, trace=False):
    import ml_dtypes

    del sum_weight  # cancels exactly in LayerNorm (shift invariance)
    nc = _get_compiled()
    x = np.asarray(x)
    if x.dtype != ml_dtypes.bfloat16:
        x = x.astype(ml_dtypes.bfloat16)
    # (N,C,D,H,W) -> (N,C,k,h,dd,w)
    xp = np.ascontiguousarray(
        x.reshape(N, C, NCHUNK, 2, H, W).transpose(0, 1, 2, 4, 3, 5)
    )
    cons = _make_consts(
        np.asarray(gamma, dtype=np.float32), np.asarray(beta, dtype=np.float32)
    )
    in_maps = []
    for core in range(NCORES):
        shard = xp[core * NPER : (core + 1) * NPER].reshape(P, NCHUNK, H, 2, W)
        # transposed copy: [(dd,w), k, h, nc]
        shT = np.ascontiguousarray(shard.transpose(3, 4, 1, 2, 0)).reshape(
            P, NCHUNK, H, P
        )
        in_maps.append(
            {
                "xs": np.ascontiguousarray(shard.reshape(P, NCHUNK, CHUNK)),
                "xsT": shT,
                "cons": cons,
            }
        )
    res = run_bass_kernel_spmd(nc, in_maps, core_ids=list(range(NCORES)), trace=trace)
    out = np.concatenate(
        [
            res.results[i]["out"]
            .astype(np.float32)
            .reshape(NPER, C, NCHUNK, HH, WW)
            for i in range(NCORES)
        ],
        axis=0,
    )
    if trace:
        return out, res
    return out


if __name__ == "__main__":
    rng = np.random.default_rng(0)
    x = rng.standard_normal((N, C, D, H, W), dtype=np.float32)
    sw = rng.standard_normal((1,)).astype(np.float32)
    gamma = rng.random((W,), dtype=np.float32)
    beta = rng.standard_normal((W,)).astype(np.float32)
    y = kernel(x, sw, gamma, beta)
    print(y.shape, y.dtype)


# revision 21
# speedup vs baseline: 1.3065x; 1.1052x over previous
"""Trainium2 Bass kernel for: x + s -> LayerNorm(W) -> 2x2x2 avgpool -> exact GELU.

Input  x: (32, 32, 16, 32, 64) f32, sum_weight (1,), gamma (64,), beta (64,)
Output:   (32, 32, 8, 16, 32) f32

Math:
  sum_weight cancels exactly (LN shift invariance).
  ln = (x - mu) * rho * gamma + beta,  rho = rsqrt(var + eps)
  8*pooled[q, w'] = S - (ga+go)[w']*M4 + 4*(be+bo)[w'] ; out = Gelu(pooled)

Performance design (v4), based on measured TRN2 rates (all vector-ish engines
run ~1 elem/ns/partition, no fast modes; DMA-transpose is descriptor-bound
and unusable at volume; PE matmuls cost ~30-80 ns each):

  * x is sent twice from the host: normal layout [nc, (k,h,dd,w)] and
    pre-transposed [(dd,w), (k,h,nc)].  The extra 8.4 MB HBM read replaces
    ~100us of on-device xbar transposes.
  * Row sums (sum x, sum x^2 over W per d-parity) run on the otherwise-idle
    PE: per h-block, stationary = xT/sqT block [128, 128nc], moving = the
    2-column dd-parity selector -> psum [nc, h, dd].  Cheap 1-bank PSUM,
    drained by a 64-elem DVE copy.
  * x^2 on ACT (square shares a table set with gelu - no table thrash).
  * DVE keeps only: xr = x*rstd (f32 out), h-pool, gamma stage, w-pair,
    +correction, batched stats math.  GPSIMD takes the f32 d-pool.
  * Correction z = bw - gw*quadsum(mu*rho) precomputed once, batched.

Layout: data-parallel over batch N (4 per core x 8 cores); partitions = 128
(n,c); chunk = one d-pair in (h, dd, w) order.
"""

import numpy as np

import concourse.bacc as bacc
import concourse.bass as bass
import concourse.tile as tile
from concourse import mybir
from concourse.bass_utils import run_bass_kernel_spmd

P = 128
N, C, D, H, W = 32, 32, 16, 32, 64
NCORES = 8
NPER = N // NCORES
EPS = 1e-5
F32 = mybir.dt.float32
BF16 = mybir.dt.bfloat16
MULT = mybir.AluOpType.mult
ADD = mybir.AluOpType.add
SUB = mybir.AluOpType.subtract

NCHUNK = D // 2  # 8
CHUNK = 2 * H * W  # 4096: (h32, dd2, w64)
ROWS = 2 * H  # 64 rows per chunk, (h, dd) order
HH = H // 2  # 16
WW = W // 2  # 32


def _kernel_body(ctx, tc: tile.TileContext, out_ap, xs, xsT, cons):
    nc = tc.nc
    stt = nc.vector.scalar_tensor_tensor
    ACTF = mybir.ActivationFunctionType

    singles = ctx.enter_context(tc.tile_pool(name="singles", bufs=1))
    statp = ctx.enter_context(tc.tile_pool(name="statp", bufs=1))
    xtp = ctx.enter_context(tc.tile_pool(name="xtp", bufs=2))
    sqp = ctx.enter_context(tc.tile_pool(name="sqp", bufs=2))
    xrp = ctx.enter_context(tc.tile_pool(name="xrp", bufs=2))
    workp = ctx.enter_context(tc.tile_pool(name="workp", bufs=2))
    outp = ctx.enter_context(tc.tile_pool(name="outp", bufs=2))

    # cons rows (bf16 [5, 128]): sel0, sel1, gamma(64), gw(32), bw(32)
    sel_t = singles.tile([P, 2], BF16)
    nc.sync.dma_start(out=sel_t[:, 0:1], in_=cons[0:1, :].rearrange("a b -> b a"))
    nc.sync.dma_start(out=sel_t[:, 1:2], in_=cons[1:2, :].rearrange("a b -> b a"))
    gam_t = singles.tile([P, W], BF16)
    nc.sync.dma_start(out=gam_t[:], in_=cons[2:3, 0:W].to_broadcast((P, W)))
    gw_t = singles.tile([P, WW], BF16)
    nc.sync.dma_start(out=gw_t[:], in_=cons[3:4, 0:WW].to_broadcast((P, WW)))
    bw_t = singles.tile([P, WW], BF16)
    nc.sync.dma_start(out=bw_t[:], in_=cons[4:5, 0:WW].to_broadcast((P, WW)))
    eps_t = singles.tile([P, 1], F32)
    nc.vector.memset(eps_t[:], EPS)

    x_all = singles.tile([P, NCHUNK, CHUNK], BF16)  # 64KB/partition
    xsf = xs.rearrange("p k f -> p (k f)")
    r1sb = statp.tile([P, NCHUNK, H, 2], F32, tag="r1")  # (k, h, dd)
    r2sb = statp.tile([P, NCHUNK, H, 2], F32, tag="r2")

    # ============ Phase A: load; PE row-sums of x and x^2 ============
    with tc.tile_pool(name="psR", space="PSUM", bufs=2) as psR, tc.tile_pool(
        name="psS", space="PSUM", bufs=2
    ) as psS:
        for k in range(NCHUNK):
            nc.sync.dma_start(
                out=x_all[:, k], in_=xsf[:, k * CHUNK : (k + 1) * CHUNK]
            )
            xT = xtp.tile([P, H, P], BF16, tag="xT")  # [(dd,w), h, nc]
            nc.sync.dma_start(out=xT[:], in_=xsT[:, k])
            sqT = sqp.tile([P, H, P], BF16, tag="sqT")
            nc.scalar.activation(
                sqT[:].rearrange("p h n -> p (h n)"),
                xT[:].rearrange("p h n -> p (h n)"),
                ACTF.Square,
            )
            pr = psR.tile([P, H, 2], F32, tag="pr")
            ps = psS.tile([P, H, 2], F32, tag="ps")
            for t in range(H):
                nc.tensor.matmul(pr[:, t, :], lhsT=xT[:, t, :], rhs=sel_t[:],
                                 start=True, stop=True)
            nc.vector.tensor_copy(out=r1sb[:, k], in_=pr[:])
            for t in range(H):
                nc.tensor.matmul(ps[:, t, :], lhsT=sqT[:, t, :], rhs=sel_t[:],
                                 start=True, stop=True)
            nc.vector.tensor_copy(out=r2sb[:, k], in_=ps[:])

    # =================== Phase B: batched stats ===================
    # 4096*var = 64*r2 - r1^2 ; rstd = rsqrt(var+eps) ; mrs = r1*rstd = 64*mu*rho
    NS = NCHUNK * ROWS  # 512
    r1f = r1sb[:].rearrange("p k h dd -> p (k h dd)")
    r2f = r2sb[:].rearrange("p k h dd -> p (k h dd)")
    s1sq = statp.tile([P, NS], F32, tag="s1sq")
    stt(out=s1sq[:], in0=r1f, scalar=1.0, in1=r1f, op0=MULT, op1=MULT)
    t64 = statp.tile([P, NS], F32, tag="t64")
    stt(out=t64[:], in0=r2f, scalar=float(W), in1=s1sq[:], op0=MULT, op1=SUB)
    rstd = statp.tile([P, NS], F32, tag="rstd")
    nc.scalar.activation(
        rstd[:], t64[:], ACTF.Sqrt, bias=eps_t[:], scale=1.0 / (W * W)
    )
    nc.vector.reciprocal(out=rstd[:], in_=rstd[:])
    mrs = statp.tile([P, NS], F32, tag="mrs")
    stt(out=mrs[:], in0=r1f, scalar=1.0, in1=rstd[:], op0=MULT, op1=MULT)
    mrs4 = mrs[:].rearrange("p (k h dd) -> p k h dd", k=NCHUNK, dd=2)
    m1 = statp.tile([P, NCHUNK, H], F32, tag="m1")
    stt(out=m1[:], in0=mrs4[:, :, :, 0], scalar=1.0, in1=mrs4[:, :, :, 1],
        op0=MULT, op1=ADD)
    m1p = m1[:].rearrange("p k (hh t) -> p k hh t", t=2)
    mq = statp.tile([P, NCHUNK, HH], F32, tag="mq")
    stt(out=mq[:], in0=m1p[:, :, :, 0], scalar=1.0, in1=m1p[:, :, :, 1],
        op0=MULT, op1=ADD)
    # z[q=(k,h'), w'] = bw - gw*mq/64  (gw scaled on host by 1/64)
    NQ = NCHUNK * HH  # 128
    mqf = mq[:].rearrange("p k h -> p (k h)")
    zneg = statp.tile([P, NQ, WW], BF16, tag="zneg")
    stt(out=zneg[:], in0=mqf.unsqueeze(2).to_broadcast((P, NQ, WW)),
        scalar=-1.0, in1=gw_t[:].unsqueeze(1).to_broadcast((P, NQ, WW)),
        op0=MULT, op1=MULT)
    z_all = statp.tile([P, NQ, WW], BF16, tag="z_all")
    stt(out=z_all[:], in0=zneg[:], scalar=1.0,
        in1=bw_t[:].unsqueeze(1).to_broadcast((P, NQ, WW)),
        op0=MULT, op1=ADD)

    # ============ Phase C: normalize + pool + gelu (elementwise) ============
    # Two chunks interleaved per step: each engine always has an independent
    # op queued, hiding cross-engine semaphore latency.  All bf16.
    outf = out_ap.rearrange("p k f -> p k f")
    PAIR = 2
    for k0 in range(0, NCHUNK, PAIR):
        ks = range(k0, k0 + PAIR)
        xrs, xds, xhs, xgs, sps, prs = {}, {}, {}, {}, {}, {}
        for k in ks:
            xc = x_all[:, k].rearrange("p (r w) -> p r w", w=W)
            xr = xrp.tile([P, ROWS, W], BF16, tag=f"xr{k % PAIR}")
            stt(out=xr[:], in0=xc, scalar=1.0,
                in1=rstd[:, k * ROWS : (k + 1) * ROWS]
                .unsqueeze(2).to_broadcast((P, ROWS, W)),
                op0=MULT, op1=MULT)
            xrs[k] = xr
        for k in ks:
            xr4 = xrs[k][:].rearrange("p (h dd) w -> p h dd w", dd=2)
            xd = workp.tile([P, H, W], BF16, tag=f"xd{k % PAIR}")
            stt(out=xd[:], in0=xr4[:, :, 0, :], scalar=1.0,
                in1=xr4[:, :, 1, :], op0=MULT, op1=ADD)
            xds[k] = xd
        for k in ks:
            xd4 = xds[k][:].rearrange("p (hh t) w -> p hh t w", t=2)
            xh = workp.tile([P, HH, W], BF16, tag=f"xh{k % PAIR}")
            stt(out=xh[:], in0=xd4[:, :, 0, :], scalar=1.0,
                in1=xd4[:, :, 1, :], op0=MULT, op1=ADD)
            xhs[k] = xh
        for k in ks:
            xg = workp.tile([P, HH, W], BF16, tag=f"xg{k % PAIR}")
            stt(out=xg[:], in0=xhs[k][:], scalar=1.0,
                in1=gam_t[:].unsqueeze(1).to_broadcast((P, HH, W)),
                op0=MULT, op1=MULT)
            xgs[k] = xg
        for k in ks:
            xg4 = xgs[k][:].rearrange("p h (v t) -> p h v t", t=2)
            spre = workp.tile([P, HH, WW], BF16, tag=f"sp{k % PAIR}")
            stt(out=spre[:], in0=xg4[:, :, :, 0], scalar=1.0,
                in1=xg4[:, :, :, 1], op0=MULT, op1=ADD)
            sps[k] = spre
        for k in ks:
            pre = workp.tile([P, HH, WW], BF16, tag=f"pr{k % PAIR}")
            stt(out=pre[:], in0=sps[k][:], scalar=1.0,
                in1=z_all[:, k * HH : (k + 1) * HH], op0=MULT, op1=ADD)
            prs[k] = pre
        for k in ks:
            res = outp.tile([P, HH * WW], BF16, tag=f"res{k % PAIR}")
            nc.scalar.activation(
                res[:], prs[k][:].rearrange("p a b -> p (a b)"),
                ACTF.Gelu, scale=0.125,
            )
            nc.sync.dma_start(out=outf[:, k], in_=res[:])


_CACHE: dict = {}


def _get_compiled():
    if "nc" not in _CACHE:
        nc = bacc.Bacc("TRN2", target_bir_lowering=False, debug=False)
        xs = nc.dram_tensor("xs", [P, NCHUNK, CHUNK], BF16, kind="ExternalInput").ap()
        xsT = nc.dram_tensor(
            "xsT", [P, NCHUNK, H, P], BF16, kind="ExternalInput"
        ).ap()
        cons = nc.dram_tensor("cons", [5, P], BF16, kind="ExternalInput").ap()
        out = nc.dram_tensor(
            "out", [P, NCHUNK, HH * WW], BF16, kind="ExternalOutput"
        ).ap()
        from contextlib import ExitStack

        with tile.TileContext(nc) as tc, ExitStack() as ctx:
            _kernel_body(ctx, tc, out, xs, xsT, cons)
        nc.compile()
        _CACHE["nc"] = nc
    return _CACHE["nc"]


def _make_consts(gamma: np.ndarray, beta: np.ndarray):
    import ml_dtypes

    cons = np.zeros((5, P), np.float32)
    dd_of_p = (np.arange(P) // W).astype(np.int32)
    cons[0] = (dd_of_p == 0).astype(np.float32)
    cons[1] = (dd_of_p == 1).astype(np.float32)
    cons[2, 0:W] = gamma
    cons[3, 0:WW] = (gamma[0::2] + gamma[1::2]) / float(W)  # mrs carries 64x
    cons[4, 0:WW] = 4.0 * (beta[0::2] + beta[1::2])
    return cons.astype(ml_dtypes.bfloat16)


def kernel(x, sum_weight, gamma, beta, trace=False):
    import ml_dtypes

    del sum_weight  # cancels exactly in LayerNorm (shift invariance)
    nc = _get_compiled()
    x = np.asarray(x)
    if x.dtype != ml_dtypes.bfloat16:
        x = x.astype(ml_dtypes.bfloat16)
    # (N,C,D,H,W) -> (N,C,k,h,dd,w)
    xp = np.ascontiguousarray(
        x.reshape(N, C, NCHUNK, 2, H, W).transpose(0, 1, 2, 4, 3, 5)
    )
    cons = _make_consts(
        np.asarray(gamma, dtype=np.float32), np.asarray(beta, dtype=np.float32)
    )
    in_maps = []
    for core in range(NCORES):
        shard = xp[core * NPER : (core + 1) * NPER].reshape(P, NCHUNK, H, 2, W)
        # transposed copy: [(dd,w), k, h, nc]
        shT = np.ascontiguousarray(shard.transpose(3, 4, 1, 2, 0)).reshape(
            P, NCHUNK, H, P
        )
        in_maps.append(
            {
                "xs": np.ascontiguousarray(shard.reshape(P, NCHUNK, CHUNK)),
                "xsT": shT,
                "cons": cons,
            }
        )
    res = run_bass_kernel_spmd(nc, in_maps, core_ids=list(range(NCORES)), trace=trace)
    out = np.concatenate(
        [
            res.results[i]["out"]
            .astype(np.float32)
            .reshape(NPER, C, NCHUNK, HH, WW)
            for i in range(NCORES)
        ],
        axis=0,
    )
    if trace:
        return out, res
    return out


if __name__ == "__main__":
    rng = np.random.default_rng(0)
    x = rng.standard_normal((N, C, D, H, W), dtype=np.float32)
    sw = rng.standard_normal((1,)).astype(np.float32)
    gamma = rng.random((W,), dtype=np.float32)
    beta = rng.standard_normal((W,)).astype(np.float32)
    y = kernel(x, sw, gamma, beta)
    print(y.shape, y.dtype)


# revision 23
# speedup vs baseline: 1.4474x; 1.1078x over previous
"""Trainium2 Bass kernel for: x + s -> LayerNorm(W) -> 2x2x2 avgpool -> exact GELU.

Input  x: (32, 32, 16, 32, 64) f32, sum_weight (1,), gamma (64,), beta (64,)
Output:   (32, 32, 8, 16, 32) f32

Math:
  sum_weight cancels exactly (LN shift invariance).
  ln = (x - mu) * rho * gamma + beta,  rho = rsqrt(var + eps)
  8*pooled[q, w'] = S - (ga+go)[w']*M4 + 4*(be+bo)[w'] ; out = Gelu(pooled)

Performance design (v4), based on measured TRN2 rates (all vector-ish engines
run ~1 elem/ns/partition, no fast modes; DMA-transpose is descriptor-bound
and unusable at volume; PE matmuls cost ~30-80 ns each):

  * x is sent twice from the host: normal layout [nc, (k,h,dd,w)] and
    pre-transposed [(dd,w), (k,h,nc)].  The extra 8.4 MB HBM read replaces
    ~100us of on-device xbar transposes.
  * Row sums (sum x, sum x^2 over W per d-parity) run on the otherwise-idle
    PE: per h-block, stationary = xT/sqT block [128, 128nc], moving = the
    2-column dd-parity selector -> psum [nc, h, dd].  Cheap 1-bank PSUM,
    drained by a 64-elem DVE copy.
  * x^2 on ACT (square shares a table set with gelu - no table thrash).
  * DVE keeps only: xr = x*rstd (f32 out), h-pool, gamma stage, w-pair,
    +correction, batched stats math.  GPSIMD takes the f32 d-pool.
  * Correction z = bw - gw*quadsum(mu*rho) precomputed once, batched.

Layout: data-parallel over batch N (4 per core x 8 cores); partitions = 128
(n,c); chunk = one d-pair in (h, dd, w) order.
"""

import numpy as np

import concourse.bacc as bacc
import concourse.bass as bass
import concourse.tile as tile
from concourse import mybir
from concourse.bass_utils import run_bass_kernel_spmd

P = 128
N, C, D, H, W = 32, 32, 16, 32, 64
NCORES = 8
NPER = N // NCORES
EPS = 1e-5
F32 = mybir.dt.float32
BF16 = mybir.dt.bfloat16
MULT = mybir.AluOpType.mult
ADD = mybir.AluOpType.add
SUB = mybir.AluOpType.subtract

NCHUNK = D // 2  # 8
CHUNK = 2 * H * W  # 4096: (h32, dd2, w64)
ROWS = 2 * H  # 64 rows per chunk, (h, dd) order
HH = H // 2  # 16
WW = W // 2  # 32


def _kernel_body(ctx, tc: tile.TileContext, out_ap, xs, xsT, cons):
    nc = tc.nc
    stt = nc.vector.scalar_tensor_tensor
    ACTF = mybir.ActivationFunctionType

    singles = ctx.enter_context(tc.tile_pool(name="singles", bufs=1))
    statp = ctx.enter_context(tc.tile_pool(name="statp", bufs=1))
    xtp = ctx.enter_context(tc.tile_pool(name="xtp", bufs=2))
    sqp = ctx.enter_context(tc.tile_pool(name="sqp", bufs=2))
    xrp = ctx.enter_context(tc.tile_pool(name="xrp", bufs=2))
    workp = ctx.enter_context(tc.tile_pool(name="workp", bufs=2))
    outp = ctx.enter_context(tc.tile_pool(name="outp", bufs=2))
    psR = ctx.enter_context(tc.tile_pool(name="psR", space="PSUM", bufs=2))
    psS = ctx.enter_context(tc.tile_pool(name="psS", space="PSUM", bufs=2))

    # cons rows (bf16 [5, 128]): sel0, sel1, gamma(64), gw(32), bw(32)
    sel_t = singles.tile([P, 2], BF16)
    nc.sync.dma_start(out=sel_t[:, 0:1], in_=cons[0:1, :].rearrange("a b -> b a"))
    nc.sync.dma_start(out=sel_t[:, 1:2], in_=cons[1:2, :].rearrange("a b -> b a"))
    gam_t = singles.tile([P, W], BF16)
    nc.sync.dma_start(out=gam_t[:], in_=cons[2:3, 0:W].to_broadcast((P, W)))
    gw_t = singles.tile([P, WW], BF16)
    nc.sync.dma_start(out=gw_t[:], in_=cons[3:4, 0:WW].to_broadcast((P, WW)))
    bw_t = singles.tile([P, WW], BF16)
    nc.sync.dma_start(out=bw_t[:], in_=cons[4:5, 0:WW].to_broadcast((P, WW)))
    eps_t = singles.tile([P, 1], F32)
    nc.vector.memset(eps_t[:], EPS)

    x_all = singles.tile([P, NCHUNK, CHUNK], BF16)  # 64KB/partition
    xsf = xs.rearrange("p k f -> p (k f)")
    r1sb = statp.tile([P, NCHUNK, H, 2], F32, tag="r1")  # (k, h, dd)
    r2sb = statp.tile([P, NCHUNK, H, 2], F32, tag="r2")
    rstd = statp.tile([P, NCHUNK * ROWS], F32, tag="rstd")
    z_all = statp.tile([P, NCHUNK * HH, WW], BF16, tag="z_all")
    outf = out_ap.rearrange("p k f -> p k f")

    def phase_a(k):
        # load; PE row-sums of x and x^2; drains on GPSIMD
        nc.sync.dma_start(out=x_all[:, k], in_=xsf[:, k * CHUNK : (k + 1) * CHUNK])
        xT = xtp.tile([P, H, P], BF16, tag="xT")  # [(dd,w), h, nc]
        nc.sync.dma_start(out=xT[:], in_=xsT[:, k])
        sqT = sqp.tile([P, H, P], BF16, tag="sqT")
        nc.scalar.activation(
            sqT[:].rearrange("p h n -> p (h n)"),
            xT[:].rearrange("p h n -> p (h n)"),
            ACTF.Square,
        )
        pr = psR.tile([P, H, 2], F32, tag="pr")
        ps = psS.tile([P, H, 2], F32, tag="ps")
        for t in range(H):
            nc.tensor.matmul(pr[:, t, :], lhsT=xT[:, t, :], rhs=sel_t[:],
                             start=True, stop=True)
        nc.scalar.copy(out=r1sb[:, k], in_=pr[:])
        for t in range(H):
            nc.tensor.matmul(ps[:, t, :], lhsT=sqT[:, t, :], rhs=sel_t[:],
                             start=True, stop=True)
        nc.scalar.copy(out=r2sb[:, k], in_=ps[:])

    HB = NCHUNK // 2  # chunks per half

    def phase_b(h):
        # batched stats for chunks [h*HB, (h+1)*HB)
        ck = slice(h * HB, (h + 1) * HB)
        NSH = HB * ROWS  # 256
        r1f = r1sb[:, ck].rearrange("p k h dd -> p (k h dd)")
        r2f = r2sb[:, ck].rearrange("p k h dd -> p (k h dd)")
        rsh = rstd[:, h * NSH : (h + 1) * NSH]
        s1sq = statp.tile([P, NSH], F32, tag=f"s1sq{h}")
        stt(out=s1sq[:], in0=r1f, scalar=1.0, in1=r1f, op0=MULT, op1=MULT)
        t64 = statp.tile([P, NSH], F32, tag=f"t64{h}")
        stt(out=t64[:], in0=r2f, scalar=float(W), in1=s1sq[:], op0=MULT, op1=SUB)
        nc.scalar.activation(
            rsh, t64[:], ACTF.Sqrt, bias=eps_t[:], scale=1.0 / (W * W)
        )
        nc.vector.reciprocal(out=rsh, in_=rsh)
        mrs = statp.tile([P, NSH], F32, tag=f"mrs{h}")
        stt(out=mrs[:], in0=r1f, scalar=1.0, in1=rsh, op0=MULT, op1=MULT)
        mrs4 = mrs[:].rearrange("p (k h dd) -> p k h dd", k=HB, dd=2)
        m1 = statp.tile([P, HB, H], F32, tag=f"m1{h}")
        stt(out=m1[:], in0=mrs4[:, :, :, 0], scalar=1.0, in1=mrs4[:, :, :, 1],
            op0=MULT, op1=ADD)
        m1p = m1[:].rearrange("p k (hh t) -> p k hh t", t=2)
        mq = statp.tile([P, HB, HH], F32, tag=f"mq{h}")
        stt(out=mq[:], in0=m1p[:, :, :, 0], scalar=1.0, in1=m1p[:, :, :, 1],
            op0=MULT, op1=ADD)
        NQH = HB * HH  # 64
        mqf = mq[:].rearrange("p k h -> p (k h)")
        zneg = statp.tile([P, NQH, WW], BF16, tag=f"zneg{h}")
        stt(out=zneg[:], in0=mqf.unsqueeze(2).to_broadcast((P, NQH, WW)),
            scalar=-1.0, in1=gw_t[:].unsqueeze(1).to_broadcast((P, NQH, WW)),
            op0=MULT, op1=MULT)
        stt(out=z_all[:, h * NQH : (h + 1) * NQH], in0=zneg[:], scalar=1.0,
            in1=bw_t[:].unsqueeze(1).to_broadcast((P, NQH, WW)),
            op0=MULT, op1=ADD)

    def phase_c(k0):
        # two chunks interleaved per step; all bf16
        ks = range(k0, k0 + 2)
        xrs, xds, xhs, xgs, sps, prs = {}, {}, {}, {}, {}, {}
        for k in ks:
            xc = x_all[:, k].rearrange("p (r w) -> p r w", w=W)
            xr = xrp.tile([P, ROWS, W], BF16, tag=f"xr{k % 2}")
            stt(out=xr[:], in0=xc, scalar=1.0,
                in1=rstd[:, k * ROWS : (k + 1) * ROWS]
                .unsqueeze(2).to_broadcast((P, ROWS, W)),
                op0=MULT, op1=MULT)
            xrs[k] = xr
        for k in ks:
            xr4 = xrs[k][:].rearrange("p (h dd) w -> p h dd w", dd=2)
            xd = workp.tile([P, H, W], BF16, tag=f"xd{k % 2}")
            stt(out=xd[:], in0=xr4[:, :, 0, :], scalar=1.0,
                in1=xr4[:, :, 1, :], op0=MULT, op1=ADD)
            xds[k] = xd
        for k in ks:
            xd4 = xds[k][:].rearrange("p (hh t) w -> p hh t w", t=2)
            xh = workp.tile([P, HH, W], BF16, tag=f"xh{k % 2}")
            stt(out=xh[:], in0=xd4[:, :, 0, :], scalar=1.0,
                in1=xd4[:, :, 1, :], op0=MULT, op1=ADD)
            xhs[k] = xh
        for k in ks:
            xg = workp.tile([P, HH, W], BF16, tag=f"xg{k % 2}")
            stt(out=xg[:], in0=xhs[k][:], scalar=1.0,
                in1=gam_t[:].unsqueeze(1).to_broadcast((P, HH, W)),
                op0=MULT, op1=MULT)
            xgs[k] = xg
        for k in ks:
            xg4 = xgs[k][:].rearrange("p h (v t) -> p h v t", t=2)
            spre = workp.tile([P, HH, WW], BF16, tag=f"sp{k % 2}")
            stt(out=spre[:], in0=xg4[:, :, :, 0], scalar=1.0,
                in1=xg4[:, :, :, 1], op0=MULT, op1=ADD)
            sps[k] = spre
        for k in ks:
            pre = workp.tile([P, HH, WW], BF16, tag=f"pr{k % 2}")
            stt(out=pre[:], in0=sps[k][:], scalar=1.0,
                in1=z_all[:, k * HH : (k + 1) * HH], op0=MULT, op1=ADD)
            prs[k] = pre
        for k in ks:
            res = outp.tile([P, HH * WW], BF16, tag=f"res{k % 2}")
            nc.scalar.activation(
                res[:], prs[k][:].rearrange("p a b -> p (a b)"),
                ACTF.Gelu, scale=0.125,
            )
            nc.sync.dma_start(out=outf[:, k], in_=res[:])

    # emission: A(first half) -> B0 -> A(second half) || C(first half) -> B1
    # -> C(second half).  A's drains ride GPSIMD so C owns the DVE queue.
    for k in range(HB):
        phase_a(k)
    phase_b(0)
    for k in range(HB, NCHUNK):
        phase_a(k)
    phase_c(0)
    phase_c(2)
    phase_b(1)
    phase_c(4)
    phase_c(6)


_CACHE: dict = {}


def _get_compiled():
    if "nc" not in _CACHE:
        nc = bacc.Bacc("TRN2", target_bir_lowering=False, debug=False)
        xs = nc.dram_tensor("xs", [P, NCHUNK, CHUNK], BF16, kind="ExternalInput").ap()
        xsT = nc.dram_tensor(
            "xsT", [P, NCHUNK, H, P], BF16, kind="ExternalInput"
        ).ap()
        cons = nc.dram_tensor("cons", [5, P], BF16, kind="ExternalInput").ap()
        out = nc.dram_tensor(
            "out", [P, NCHUNK, HH * WW], BF16, kind="ExternalOutput"
        ).ap()
        from contextlib import ExitStack

        with tile.TileContext(nc) as tc, ExitStack() as ctx:
            _kernel_body(ctx, tc, out, xs, xsT, cons)
        nc.compile()
        _CACHE["nc"] = nc
    return _CACHE["nc"]


def _make_consts(gamma: np.ndarray, beta: np.ndarray):
    import ml_dtypes

    cons = np.zeros((5, P), np.float32)
    dd_of_p = (np.arange(P) // W).astype(np.int32)
    cons[0] = (dd_of_p == 0).astype(np.float32)
    cons[1] = (dd_of_p == 1).astype(np.float32)
    cons[2, 0:W] = gamma
    cons[3, 0:WW] = (gamma[0::2] + gamma[1::2]) / float(W)  # mrs carries 64x
    cons[4, 0:WW] = 4.0 * (beta[0::2] + beta[1::2])
    return cons.astype(ml_dtypes.bfloat16)


def kernel(x, sum_weight, gamma, beta, trace=False):
    import ml_dtypes

    del sum_weight  # cancels exactly in LayerNorm (shift invariance)
    nc = _get_compiled()
    x = np.asarray(x)
    if x.dtype != ml_dtypes.bfloat16:
        x = x.astype(ml_dtypes.bfloat16)
    # (N,C,D,H,W) -> (N,C,k,h,dd,w)
    xp = np.ascontiguousarray(
        x.reshape(N, C, NCHUNK, 2, H, W).transpose(0, 1, 2, 4, 3, 5)
    )
    cons = _make_consts(
        np.asarray(gamma, dtype=np.float32), np.asarray(beta, dtype=np.float32)
    )
    in_maps = []
    for core in range(NCORES):
        shard = xp[core * NPER : (core + 1) * NPER].reshape(P, NCHUNK, H, 2, W)
        # transposed copy: [(dd,w), k, h, nc]
        shT = np.ascontiguousarray(shard.transpose(3, 4, 1, 2, 0)).reshape(
            P, NCHUNK, H, P
        )
        in_maps.append(
            {
                "xs": np.ascontiguousarray(shard.reshape(P, NCHUNK, CHUNK)),
                "xsT": shT,
                "cons": cons,
            }
        )
    res = run_bass_kernel_spmd(nc, in_maps, core_ids=list(range(NCORES)), trace=trace)
    out = np.concatenate(
        [
            res.results[i]["out"]
            .astype(np.float32)
            .reshape(NPER, C, NCHUNK, HH, WW)
            for i in range(NCORES)
        ],
        axis=0,
    )
    if trace:
        return out, res
    return out


if __name__ == "__main__":
    rng = np.random.default_rng(0)
    x = rng.standard_normal((N, C, D, H, W), dtype=np.float32)
    sw = rng.standard_normal((1,)).astype(np.float32)
    gamma = rng.random((W,), dtype=np.float32)
    beta = rng.standard_normal((W,)).astype(np.float32)
    y = kernel(x, sw, gamma, beta)
    print(y.shape, y.dtype)


# revision 26
# speedup vs baseline: 1.4719x; 1.0170x over previous
"""Trainium2 Bass kernel for: x + s -> LayerNorm(W) -> 2x2x2 avgpool -> exact GELU.

Input  x: (32, 32, 16, 32, 64) f32, sum_weight (1,), gamma (64,), beta (64,)
Output:   (32, 32, 8, 16, 32) f32

Math:
  sum_weight cancels exactly (LN shift invariance).
  ln = (x - mu) * rho * gamma + beta,  rho = rsqrt(var + eps)
  8*pooled[q, w'] = S - (ga+go)[w']*M4 + 4*(be+bo)[w'] ; out = Gelu(pooled)

Performance design (v4), based on measured TRN2 rates (all vector-ish engines
run ~1 elem/ns/partition, no fast modes; DMA-transpose is descriptor-bound
and unusable at volume; PE matmuls cost ~30-80 ns each):

  * x is sent twice from the host: normal layout [nc, (k,h,dd,w)] and
    pre-transposed [(dd,w), (k,h,nc)].  The extra 8.4 MB HBM read replaces
    ~100us of on-device xbar transposes.
  * Row sums (sum x, sum x^2 over W per d-parity) run on the otherwise-idle
    PE: per h-block, stationary = xT/sqT block [128, 128nc], moving = the
    2-column dd-parity selector -> psum [nc, h, dd].  Cheap 1-bank PSUM,
    drained by a 64-elem DVE copy.
  * x^2 on ACT (square shares a table set with gelu - no table thrash).
  * DVE keeps only: xr = x*rstd (f32 out), h-pool, gamma stage, w-pair,
    +correction, batched stats math.  GPSIMD takes the f32 d-pool.
  * Correction z = bw - gw*quadsum(mu*rho) precomputed once, batched.

Layout: data-parallel over batch N (4 per core x 8 cores); partitions = 128
(n,c); chunk = one d-pair in (h, dd, w) order.
"""

import numpy as np

import concourse.bacc as bacc
import concourse.bass as bass
import concourse.tile as tile
from concourse import mybir
from concourse.bass_utils import run_bass_kernel_spmd

P = 128
N, C, D, H, W = 32, 32, 16, 32, 64
NCORES = 8
NPER = N // NCORES
EPS = 1e-5
F32 = mybir.dt.float32
BF16 = mybir.dt.bfloat16
MULT = mybir.AluOpType.mult
ADD = mybir.AluOpType.add
SUB = mybir.AluOpType.subtract

NCHUNK = D // 2  # 8
CHUNK = 2 * H * W  # 4096: (h32, dd2, w64)
ROWS = 2 * H  # 64 rows per chunk, (h, dd) order
HH = H // 2  # 16
WW = W // 2  # 32


def _kernel_body(ctx, tc: tile.TileContext, out_ap, xs, xsT, cons):
    nc = tc.nc
    stt = nc.vector.scalar_tensor_tensor
    ACTF = mybir.ActivationFunctionType

    singles = ctx.enter_context(tc.tile_pool(name="singles", bufs=1))
    statp = ctx.enter_context(tc.tile_pool(name="statp", bufs=1))
    xtp = ctx.enter_context(tc.tile_pool(name="xtp", bufs=2))
    sqp = ctx.enter_context(tc.tile_pool(name="sqp", bufs=2))
    xrp = ctx.enter_context(tc.tile_pool(name="xrp", bufs=2))
    workp = ctx.enter_context(tc.tile_pool(name="workp", bufs=2))
    outp = ctx.enter_context(tc.tile_pool(name="outp", bufs=2))
    psR = ctx.enter_context(tc.tile_pool(name="psR", space="PSUM", bufs=2))
    psS = ctx.enter_context(tc.tile_pool(name="psS", space="PSUM", bufs=2))

    # cons rows (bf16 [5, 128]): sel0, sel1, gamma(64), gw(32), bw(32)
    sel_t = singles.tile([P, 2], BF16)
    nc.sync.dma_start(out=sel_t[:, 0:1], in_=cons[0:1, :].rearrange("a b -> b a"))
    nc.sync.dma_start(out=sel_t[:, 1:2], in_=cons[1:2, :].rearrange("a b -> b a"))
    gam_t = singles.tile([P, W], BF16)
    nc.sync.dma_start(out=gam_t[:], in_=cons[2:3, 0:W].to_broadcast((P, W)))
    gw_t = singles.tile([P, WW], BF16)
    nc.sync.dma_start(out=gw_t[:], in_=cons[3:4, 0:WW].to_broadcast((P, WW)))
    bw_t = singles.tile([P, WW], BF16)
    nc.sync.dma_start(out=bw_t[:], in_=cons[4:5, 0:WW].to_broadcast((P, WW)))
    eps_t = singles.tile([P, 1], F32)
    nc.vector.memset(eps_t[:], EPS)
    inv64_t = singles.tile([P, 1], F32)
    nc.vector.memset(inv64_t[:], 1.0 / W)

    x_all = singles.tile([P, NCHUNK, CHUNK], BF16)  # 64KB/partition
    xsf = xs.rearrange("p k f -> p (k f)")
    r1sb = statp.tile([P, NCHUNK, H, 2], F32, tag="r1")  # (k, h, dd)
    r2sb = statp.tile([P, NCHUNK, H, 2], F32, tag="r2")
    rstd = statp.tile([P, NCHUNK * ROWS], F32, tag="rstd")
    z_all = statp.tile([P, NCHUNK * HH, WW], BF16, tag="z_all")
    outf = out_ap.rearrange("p k f -> p k f")

    def phase_a(k):
        # load; PE row-sums of x and x^2; drains on GPSIMD
        nc.sync.dma_start(out=x_all[:, k], in_=xsf[:, k * CHUNK : (k + 1) * CHUNK])
        xT = xtp.tile([P, H, P], BF16, tag="xT")  # [(dd,w), h, nc]
        nc.sync.dma_start(out=xT[:], in_=xsT[:, k])
        sqT = sqp.tile([P, H, P], BF16, tag="sqT")
        nc.scalar.activation(
            sqT[:].rearrange("p h n -> p (h n)"),
            xT[:].rearrange("p h n -> p (h n)"),
            ACTF.Square,
        )
        pr = psR.tile([P, H, 2], F32, tag="pr")
        ps = psS.tile([P, H, 2], F32, tag="ps")
        for t in range(H):
            nc.tensor.matmul(pr[:, t, :], lhsT=xT[:, t, :], rhs=sel_t[:],
                             start=True, stop=True)
        nc.scalar.copy(out=r1sb[:, k], in_=pr[:])
        for t in range(H):
            nc.tensor.matmul(ps[:, t, :], lhsT=sqT[:, t, :], rhs=sel_t[:],
                             start=True, stop=True)
        nc.scalar.copy(out=r2sb[:, k], in_=ps[:])

    HB = NCHUNK // 2  # chunks per half

    def phase_b(h):
        # batched stats for chunks [h*HB, (h+1)*HB)
        ck = slice(h * HB, (h + 1) * HB)
        NSH = HB * ROWS  # 256
        r1f = r1sb[:, ck].rearrange("p k h dd -> p (k h dd)")
        r2f = r2sb[:, ck].rearrange("p k h dd -> p (k h dd)")
        rsh = rstd[:, h * NSH : (h + 1) * NSH]
        # var path on GPSIMD: t = r2 - (r1/64)*r1  (= 64*var)
        ra = statp.tile([P, NSH], F32, tag=f"ra{h}")
        nc.gpsimd.tensor_mul(ra[:], r1f, inv64_t[:].to_broadcast((P, NSH)))
        s1sq = statp.tile([P, NSH], F32, tag=f"s1sq{h}")
        nc.gpsimd.tensor_mul(s1sq[:], ra[:], r1f)
        t64 = statp.tile([P, NSH], F32, tag=f"t64{h}")
        nc.gpsimd.tensor_sub(t64[:], r2f, s1sq[:])
        sqv = statp.tile([P, NSH], F32, tag=f"sqv{h}")
        nc.scalar.activation(
            sqv[:], t64[:], ACTF.Sqrt, bias=eps_t[:], scale=1.0 / W
        )
        nc.vector.reciprocal_approx_fast(out=rsh, in_=sqv[:])
        mrs = statp.tile([P, NSH], F32, tag=f"mrs{h}")
        nc.gpsimd.tensor_mul(mrs[:], r1f, rsh)
        mrs4 = mrs[:].rearrange("p (k h dd) -> p k h dd", k=HB, dd=2)
        m1 = statp.tile([P, HB, H], F32, tag=f"m1{h}")
        nc.gpsimd.tensor_add(m1[:], mrs4[:, :, :, 0], mrs4[:, :, :, 1])
        m1p = m1[:].rearrange("p k (hh t) -> p k hh t", t=2)
        mq = statp.tile([P, HB, HH], F32, tag=f"mq{h}")
        nc.gpsimd.tensor_add(mq[:], m1p[:, :, :, 0], m1p[:, :, :, 1])
        NQH = HB * HH  # 64
        mqf = mq[:].rearrange("p k h -> p (k h)")
        zneg = statp.tile([P, NQH, WW], BF16, tag=f"zneg{h}")
        stt(out=zneg[:], in0=mqf.unsqueeze(2).to_broadcast((P, NQH, WW)),
            scalar=-1.0, in1=gw_t[:].unsqueeze(1).to_broadcast((P, NQH, WW)),
            op0=MULT, op1=MULT)
        stt(out=z_all[:, h * NQH : (h + 1) * NQH], in0=zneg[:], scalar=1.0,
            in1=bw_t[:].unsqueeze(1).to_broadcast((P, NQH, WW)),
            op0=MULT, op1=ADD)

    def phase_c(k0):
        # two chunks interleaved per step; all bf16
        ks = range(k0, k0 + 2)
        xrs, xds, xhs, xgs, sps, prs = {}, {}, {}, {}, {}, {}
        for k in ks:
            xc = x_all[:, k].rearrange("p (r w) -> p r w", w=W)
            xr = xrp.tile([P, ROWS, W], BF16, tag=f"xr{k % 2}")
            stt(out=xr[:], in0=xc, scalar=1.0,
                in1=rstd[:, k * ROWS : (k + 1) * ROWS]
                .unsqueeze(2).to_broadcast((P, ROWS, W)),
                op0=MULT, op1=MULT)
            xrs[k] = xr
        for k in ks:
            xr4 = xrs[k][:].rearrange("p (h dd) w -> p h dd w", dd=2)
            xd = workp.tile([P, H, W], BF16, tag=f"xd{k % 2}")
            stt(out=xd[:], in0=xr4[:, :, 0, :], scalar=1.0,
                in1=xr4[:, :, 1, :], op0=MULT, op1=ADD)
            xds[k] = xd
        for k in ks:
            xd4 = xds[k][:].rearrange("p (hh t) w -> p hh t w", t=2)
            xh = workp.tile([P, HH, W], BF16, tag=f"xh{k % 2}")
            stt(out=xh[:], in0=xd4[:, :, 0, :], scalar=1.0,
                in1=xd4[:, :, 1, :], op0=MULT, op1=ADD)
            xhs[k] = xh
        for k in ks:
            xg = workp.tile([P, HH, W], BF16, tag=f"xg{k % 2}")
            stt(out=xg[:], in0=xhs[k][:], scalar=1.0,
                in1=gam_t[:].unsqueeze(1).to_broadcast((P, HH, W)),
                op0=MULT, op1=MULT)
            xgs[k] = xg
        for k in ks:
            xg4 = xgs[k][:].rearrange("p h (v t) -> p h v t", t=2)
            spre = workp.tile([P, HH, WW], BF16, tag=f"sp{k % 2}")
            stt(out=spre[:], in0=xg4[:, :, :, 0], scalar=1.0,
                in1=xg4[:, :, :, 1], op0=MULT, op1=ADD)
            sps[k] = spre
        for k in ks:
            pre = workp.tile([P, HH, WW], BF16, tag=f"pr{k % 2}")
            stt(out=pre[:], in0=sps[k][:], scalar=1.0,
                in1=z_all[:, k * HH : (k + 1) * HH], op0=MULT, op1=ADD)
            prs[k] = pre
        for k in ks:
            res = outp.tile([P, HH * WW], BF16, tag=f"res{k % 2}")
            nc.scalar.activation(
                res[:], prs[k][:].rearrange("p a b -> p (a b)"),
                ACTF.Gelu, scale=0.125,
            )
            nc.sync.dma_start(out=outf[:, k], in_=res[:])

    # emission: A(first half) -> B0 -> A(second half) || C(first half) -> B1
    # -> C(second half).  A's drains ride GPSIMD so C owns the DVE queue.
    for k in range(HB):
        phase_a(k)
    phase_b(0)
    for k in range(HB, NCHUNK):
        phase_a(k)
    phase_c(0)
    phase_c(2)
    phase_b(1)
    phase_c(4)
    phase_c(6)


_CACHE: dict = {}


def _get_compiled():
    if "nc" not in _CACHE:
        nc = bacc.Bacc("TRN2", target_bir_lowering=False, debug=False)
        xs = nc.dram_tensor("xs", [P, NCHUNK, CHUNK], BF16, kind="ExternalInput").ap()
        xsT = nc.dram_tensor(
            "xsT", [P, NCHUNK, H, P], BF16, kind="ExternalInput"
        ).ap()
        cons = nc.dram_tensor("cons", [5, P], BF16, kind="ExternalInput").ap()
        out = nc.dram_tensor(
            "out", [P, NCHUNK, HH * WW], BF16, kind="ExternalOutput"
        ).ap()
        from contextlib import ExitStack

        with tile.TileContext(nc) as tc, ExitStack() as ctx:
            _kernel_body(ctx, tc, out, xs, xsT, cons)
        nc.compile()
        _CACHE["nc"] = nc
    return _CACHE["nc"]


def _make_consts(gamma: np.ndarray, beta: np.ndarray):
    import ml_dtypes

    cons = np.zeros((5, P), np.float32)
    dd_of_p = (np.arange(P) // W).astype(np.int32)
    cons[0] = (dd_of_p == 0).astype(np.float32)
    cons[1] = (dd_of_p == 1).astype(np.float32)
    cons[2, 0:W] = gamma
    cons[3, 0:WW] = (gamma[0::2] + gamma[1::2]) / float(W)  # mrs carries 64x
    cons[4, 0:WW] = 4.0 * (beta[0::2] + beta[1::2])
    return cons.astype(ml_dtypes.bfloat16)


def kernel(x, sum_weight, gamma, beta, trace=False):
    import ml_dtypes

    del sum_weight  # cancels exactly in LayerNorm (shift invariance)
    nc = _get_compiled()
    x = np.asarray(x)
    if x.dtype != ml_dtypes.bfloat16:
        x = x.astype(ml_dtypes.bfloat16)
    # (N,C,D,H,W) -> (N,C,k,h,dd,w)
    xp = np.ascontiguousarray(
        x.reshape(N, C, NCHUNK, 2, H, W).transpose(0, 1, 2, 4, 3, 5)
    )
    cons = _make_consts(
        np.asarray(gamma, dtype=np.float32), np.asarray(beta, dtype=np.float32)
    )
    in_maps = []
    for core in range(NCORES):
        shard = xp[core * NPER : (core + 1) * NPER].reshape(P, NCHUNK, H, 2, W)
        # transposed copy: [(dd,w), k, h, nc]
        shT = np.ascontiguousarray(shard.transpose(3, 4, 1, 2, 0)).reshape(
            P, NCHUNK, H, P
        )
        in_maps.append(
            {
                "xs": np.ascontiguousarray(shard.reshape(P, NCHUNK, CHUNK)),
                "xsT": shT,
                "cons": cons,
            }
        )
    res = run_bass_kernel_spmd(nc, in_maps, core_ids=list(range(NCORES)), trace=trace)
    out = np.concatenate(
        [
            res.results[i]["out"]
            .astype(np.float32)
            .reshape(NPER, C, NCHUNK, HH, WW)
            for i in range(NCORES)
        ],
        axis=0,
    )
    if trace:
        return out, res
    return out


if __name__ == "__main__":
    rng = np.random.default_rng(0)
    x = rng.standard_normal((N, C, D, H, W), dtype=np.float32)
    sw = rng.standard_normal((1,)).astype(np.float32)
    gamma = rng.random((W,), dtype=np.float32)
    beta = rng.standard_normal((W,)).astype(np.float32)
    y = kernel(x, sw, gamma, beta)
    print(y.shape, y.dtype)


# revision 27
# speedup vs baseline: 1.4953x; 1.0159x over previous
"""Trainium2 Bass kernel for: x + s -> LayerNorm(W) -> 2x2x2 avgpool -> exact GELU.

Input  x: (32, 32, 16, 32, 64) f32, sum_weight (1,), gamma (64,), beta (64,)
Output:   (32, 32, 8, 16, 32) f32

Math:
  sum_weight cancels exactly (LN shift invariance).
  ln = (x - mu) * rho * gamma + beta,  rho = rsqrt(var + eps)
  8*pooled[q, w'] = S - (ga+go)[w']*M4 + 4*(be+bo)[w'] ; out = Gelu(pooled)

Performance design (v4), based on measured TRN2 rates (all vector-ish engines
run ~1 elem/ns/partition, no fast modes; DMA-transpose is descriptor-bound
and unusable at volume; PE matmuls cost ~30-80 ns each):

  * x is sent twice from the host: normal layout [nc, (k,h,dd,w)] and
    pre-transposed [(dd,w), (k,h,nc)].  The extra 8.4 MB HBM read replaces
    ~100us of on-device xbar transposes.
  * Row sums (sum x, sum x^2 over W per d-parity) run on the otherwise-idle
    PE: per h-block, stationary = xT/sqT block [128, 128nc], moving = the
    2-column dd-parity selector -> psum [nc, h, dd].  Cheap 1-bank PSUM,
    drained by a 64-elem DVE copy.
  * x^2 on ACT (square shares a table set with gelu - no table thrash).
  * DVE keeps only: xr = x*rstd (f32 out), h-pool, gamma stage, w-pair,
    +correction, batched stats math.  GPSIMD takes the f32 d-pool.
  * Correction z = bw - gw*quadsum(mu*rho) precomputed once, batched.

Layout: data-parallel over batch N (4 per core x 8 cores); partitions = 128
(n,c); chunk = one d-pair in (h, dd, w) order.
"""

import numpy as np

import concourse.bacc as bacc
import concourse.bass as bass
import concourse.tile as tile
from concourse import mybir
from concourse.bass_utils import run_bass_kernel_spmd

P = 128
N, C, D, H, W = 32, 32, 16, 32, 64
NCORES = 8
NPER = N // NCORES
EPS = 1e-5
F32 = mybir.dt.float32
BF16 = mybir.dt.bfloat16
MULT = mybir.AluOpType.mult
ADD = mybir.AluOpType.add
SUB = mybir.AluOpType.subtract

NCHUNK = D // 2  # 8
CHUNK = 2 * H * W  # 4096: (h32, dd2, w64)
ROWS = 2 * H  # 64 rows per chunk, (h, dd) order
HH = H // 2  # 16
WW = W // 2  # 32


def _kernel_body(ctx, tc: tile.TileContext, out_ap, xs, xsT, cons):
    nc = tc.nc
    stt = nc.vector.scalar_tensor_tensor
    ACTF = mybir.ActivationFunctionType

    singles = ctx.enter_context(tc.tile_pool(name="singles", bufs=1))
    statp = ctx.enter_context(tc.tile_pool(name="statp", bufs=1))
    xtp = ctx.enter_context(tc.tile_pool(name="xtp", bufs=2))
    sqp = ctx.enter_context(tc.tile_pool(name="sqp", bufs=2))
    xrp = ctx.enter_context(tc.tile_pool(name="xrp", bufs=1))
    workp = ctx.enter_context(tc.tile_pool(name="workp", bufs=1))
    outp = ctx.enter_context(tc.tile_pool(name="outp", bufs=1))
    psR = ctx.enter_context(tc.tile_pool(name="psR", space="PSUM", bufs=2))
    psS = ctx.enter_context(tc.tile_pool(name="psS", space="PSUM", bufs=2))

    # cons rows (bf16 [5, 128]): sel0, sel1, gamma(64), gw(32), bw(32)
    sel_t = singles.tile([P, 2], BF16)
    nc.sync.dma_start(out=sel_t[:, 0:1], in_=cons[0:1, :].rearrange("a b -> b a"))
    nc.sync.dma_start(out=sel_t[:, 1:2], in_=cons[1:2, :].rearrange("a b -> b a"))
    gam_t = singles.tile([P, W], BF16)
    nc.sync.dma_start(out=gam_t[:], in_=cons[2:3, 0:W].to_broadcast((P, W)))
    gw_t = singles.tile([P, WW], BF16)
    nc.sync.dma_start(out=gw_t[:], in_=cons[3:4, 0:WW].to_broadcast((P, WW)))
    bw_t = singles.tile([P, WW], BF16)
    nc.sync.dma_start(out=bw_t[:], in_=cons[4:5, 0:WW].to_broadcast((P, WW)))
    eps_t = singles.tile([P, 1], F32)
    nc.vector.memset(eps_t[:], EPS)
    inv64_t = singles.tile([P, 1], F32)
    nc.vector.memset(inv64_t[:], 1.0 / W)

    x_all = singles.tile([P, NCHUNK, CHUNK], BF16)  # 64KB/partition
    xsf = xs.rearrange("p k f -> p (k f)")
    r1sb = statp.tile([P, NCHUNK, H, 2], F32, tag="r1")  # (k, h, dd)
    r2sb = statp.tile([P, NCHUNK, H, 2], F32, tag="r2")
    rstd = statp.tile([P, NCHUNK * ROWS], F32, tag="rstd")
    z_all = statp.tile([P, NCHUNK * HH, WW], BF16, tag="z_all")
    outf = out_ap.rearrange("p k f -> p k f")

    def phase_a(k):
        # load; PE row-sums of x and x^2; drains on GPSIMD
        nc.sync.dma_start(out=x_all[:, k], in_=xsf[:, k * CHUNK : (k + 1) * CHUNK])
        xT = xtp.tile([P, H, P], BF16, tag="xT")  # [(dd,w), h, nc]
        nc.sync.dma_start(out=xT[:], in_=xsT[:, k])
        sqT = sqp.tile([P, H, P], BF16, tag="sqT")
        nc.scalar.activation(
            sqT[:].rearrange("p h n -> p (h n)"),
            xT[:].rearrange("p h n -> p (h n)"),
            ACTF.Square,
        )
        pr = psR.tile([P, H, 2], F32, tag="pr")
        ps = psS.tile([P, H, 2], F32, tag="ps")
        for t in range(H):
            nc.tensor.matmul(pr[:, t, :], lhsT=xT[:, t, :], rhs=sel_t[:],
                             start=True, stop=True)
        nc.scalar.copy(out=r1sb[:, k], in_=pr[:])
        for t in range(H):
            nc.tensor.matmul(ps[:, t, :], lhsT=sqT[:, t, :], rhs=sel_t[:],
                             start=True, stop=True)
        nc.scalar.copy(out=r2sb[:, k], in_=ps[:])

    HB = NCHUNK // 2  # chunks per half

    def phase_b(h):
        # batched stats for chunks [h*HB, (h+1)*HB)
        ck = slice(h * HB, (h + 1) * HB)
        NSH = HB * ROWS  # 256
        r1f = r1sb[:, ck].rearrange("p k h dd -> p (k h dd)")
        r2f = r2sb[:, ck].rearrange("p k h dd -> p (k h dd)")
        rsh = rstd[:, h * NSH : (h + 1) * NSH]
        # var path on GPSIMD: t = r2 - (r1/64)*r1  (= 64*var)
        ra = statp.tile([P, NSH], F32, tag=f"ra{h}")
        nc.gpsimd.tensor_mul(ra[:], r1f, inv64_t[:].to_broadcast((P, NSH)))
        s1sq = statp.tile([P, NSH], F32, tag=f"s1sq{h}")
        nc.gpsimd.tensor_mul(s1sq[:], ra[:], r1f)
        t64 = statp.tile([P, NSH], F32, tag=f"t64{h}")
        nc.gpsimd.tensor_sub(t64[:], r2f, s1sq[:])
        sqv = statp.tile([P, NSH], F32, tag=f"sqv{h}")
        nc.scalar.activation(
            sqv[:], t64[:], ACTF.Sqrt, bias=eps_t[:], scale=1.0 / W
        )
        nc.vector.reciprocal_approx_fast(out=rsh, in_=sqv[:])
        mrs = statp.tile([P, NSH], F32, tag=f"mrs{h}")
        nc.gpsimd.tensor_mul(mrs[:], r1f, rsh)
        mrs4 = mrs[:].rearrange("p (k h dd) -> p k h dd", k=HB, dd=2)
        m1 = statp.tile([P, HB, H], F32, tag=f"m1{h}")
        nc.gpsimd.tensor_add(m1[:], mrs4[:, :, :, 0], mrs4[:, :, :, 1])
        m1p = m1[:].rearrange("p k (hh t) -> p k hh t", t=2)
        mq = statp.tile([P, HB, HH], F32, tag=f"mq{h}")
        nc.gpsimd.tensor_add(mq[:], m1p[:, :, :, 0], m1p[:, :, :, 1])
        NQH = HB * HH  # 64
        mqf = mq[:].rearrange("p k h -> p (k h)")
        zneg = statp.tile([P, NQH, WW], BF16, tag=f"zneg{h}")
        stt(out=zneg[:], in0=mqf.unsqueeze(2).to_broadcast((P, NQH, WW)),
            scalar=-1.0, in1=gw_t[:].unsqueeze(1).to_broadcast((P, NQH, WW)),
            op0=MULT, op1=MULT)
        stt(out=z_all[:, h * NQH : (h + 1) * NQH], in0=zneg[:], scalar=1.0,
            in1=bw_t[:].unsqueeze(1).to_broadcast((P, NQH, WW)),
            op0=MULT, op1=ADD)

    def phase_c(k0):
        # four chunks interleaved per step; all bf16
        ks = range(k0, k0 + 4)
        xrs, xds, xhs, xgs, sps, prs = {}, {}, {}, {}, {}, {}
        for k in ks:
            xc = x_all[:, k].rearrange("p (r w) -> p r w", w=W)
            xr = xrp.tile([P, ROWS, W], BF16, tag=f"xr{k % 4}")
            stt(out=xr[:], in0=xc, scalar=1.0,
                in1=rstd[:, k * ROWS : (k + 1) * ROWS]
                .unsqueeze(2).to_broadcast((P, ROWS, W)),
                op0=MULT, op1=MULT)
            xrs[k] = xr
        for k in ks:
            xr4 = xrs[k][:].rearrange("p (h dd) w -> p h dd w", dd=2)
            xd = workp.tile([P, H, W], BF16, tag=f"xd{k % 4}")
            stt(out=xd[:], in0=xr4[:, :, 0, :], scalar=1.0,
                in1=xr4[:, :, 1, :], op0=MULT, op1=ADD)
            xds[k] = xd
        for k in ks:
            xd4 = xds[k][:].rearrange("p (hh t) w -> p hh t w", t=2)
            xh = workp.tile([P, HH, W], BF16, tag=f"xh{k % 4}")
            stt(out=xh[:], in0=xd4[:, :, 0, :], scalar=1.0,
                in1=xd4[:, :, 1, :], op0=MULT, op1=ADD)
            xhs[k] = xh
        for k in ks:
            xg = workp.tile([P, HH, W], BF16, tag=f"xg{k % 4}")
            stt(out=xg[:], in0=xhs[k][:], scalar=1.0,
                in1=gam_t[:].unsqueeze(1).to_broadcast((P, HH, W)),
                op0=MULT, op1=MULT)
            xgs[k] = xg
        for k in ks:
            xg4 = xgs[k][:].rearrange("p h (v t) -> p h v t", t=2)
            spre = workp.tile([P, HH, WW], BF16, tag=f"sp{k % 4}")
            stt(out=spre[:], in0=xg4[:, :, :, 0], scalar=1.0,
                in1=xg4[:, :, :, 1], op0=MULT, op1=ADD)
            sps[k] = spre
        for k in ks:
            pre = workp.tile([P, HH, WW], BF16, tag=f"pr{k % 4}")
            stt(out=pre[:], in0=sps[k][:], scalar=1.0,
                in1=z_all[:, k * HH : (k + 1) * HH], op0=MULT, op1=ADD)
            prs[k] = pre
        for k in ks:
            res = outp.tile([P, HH * WW], BF16, tag=f"res{k % 4}")
            nc.scalar.activation(
                res[:], prs[k][:].rearrange("p a b -> p (a b)"),
                ACTF.Gelu, scale=0.125,
            )
            nc.sync.dma_start(out=outf[:, k], in_=res[:])

    # emission: A(first half) -> B0 -> A(second half) || C(first half) -> B1
    # -> C(second half).  A's drains ride GPSIMD so C owns the DVE queue.
    for k in range(HB):
        phase_a(k)
    phase_b(0)
    for k in range(HB, NCHUNK):
        phase_a(k)
    phase_c(0)
    phase_b(1)
    phase_c(4)


_CACHE: dict = {}


def _get_compiled():
    if "nc" not in _CACHE:
        nc = bacc.Bacc("TRN2", target_bir_lowering=False, debug=False)
        xs = nc.dram_tensor("xs", [P, NCHUNK, CHUNK], BF16, kind="ExternalInput").ap()
        xsT = nc.dram_tensor(
            "xsT", [P, NCHUNK, H, P], BF16, kind="ExternalInput"
        ).ap()
        cons = nc.dram_tensor("cons", [5, P], BF16, kind="ExternalInput").ap()
        out = nc.dram_tensor(
            "out", [P, NCHUNK, HH * WW], BF16, kind="ExternalOutput"
        ).ap()
        from contextlib import ExitStack

        with tile.TileContext(nc) as tc, ExitStack() as ctx:
            _kernel_body(ctx, tc, out, xs, xsT, cons)
        nc.compile()
        _CACHE["nc"] = nc
    return _CACHE["nc"]


def _make_consts(gamma: np.ndarray, beta: np.ndarray):
    import ml_dtypes

    cons = np.zeros((5, P), np.float32)
    dd_of_p = (np.arange(P) // W).astype(np.int32)
    cons[0] = (dd_of_p == 0).astype(np.float32)
    cons[1] = (dd_of_p == 1).astype(np.float32)
    cons[2, 0:W] = gamma
    cons[3, 0:WW] = (gamma[0::2] + gamma[1::2]) / float(W)  # mrs carries 64x
    cons[4, 0:WW] = 4.0 * (beta[0::2] + beta[1::2])
    return cons.astype(ml_dtypes.bfloat16)


def kernel(x, sum_weight, gamma, beta, trace=False):
    import ml_dtypes

    del sum_weight  # cancels exactly in LayerNorm (shift invariance)
    nc = _get_compiled()
    x = np.asarray(x)
    if x.dtype != ml_dtypes.bfloat16:
        x = x.astype(ml_dtypes.bfloat16)
    # (N,C,D,H,W) -> (N,C,k,h,dd,w)
    xp = np.ascontiguousarray(
        x.reshape(N, C, NCHUNK, 2, H, W).transpose(0, 1, 2, 4, 3, 5)
    )
    cons = _make_consts(
        np.asarray(gamma, dtype=np.float32), np.asarray(beta, dtype=np.float32)
    )
    in_maps = []
    for core in range(NCORES):
        shard = xp[core * NPER : (core + 1) * NPER].reshape(P, NCHUNK, H, 2, W)
        # transposed copy: [(dd,w), k, h, nc]
        shT = np.ascontiguousarray(shard.transpose(3, 4, 1, 2, 0)).reshape(
            P, NCHUNK, H, P
        )
        in_maps.append(
            {
                "xs": np.ascontiguousarray(shard.reshape(P, NCHUNK, CHUNK)),
                "xsT": shT,
                "cons": cons,
            }
        )
    res = run_bass_kernel_spmd(nc, in_maps, core_ids=list(range(NCORES)), trace=trace)
    out = np.concatenate(
        [
            res.results[i]["out"]
            .astype(np.float32)
            .reshape(NPER, C, NCHUNK, HH, WW)
            for i in range(NCORES)
        ],
        axis=0,
    )
    if trace:
        return out, res
    return out


if __name__ == "__main__":
    rng = np.random.default_rng(0)
    x = rng.standard_normal((N, C, D, H, W), dtype=np.float32)
    sw = rng.standard_normal((1,)).astype(np.float32)
    gamma = rng.random((W,), dtype=np.float32)
    beta = rng.standard_normal((W,)).astype(np.float32)
    y = kernel(x, sw, gamma, beta)
    print(y.shape, y.dtype)


# revision 28
# speedup vs baseline: 1.5983x; 1.0689x over previous
"""Trainium2 Bass kernel for: x + s -> LayerNorm(W) -> 2x2x2 avgpool -> exact GELU.

Input  x: (32, 32, 16, 32, 64) f32, sum_weight (1,), gamma (64,), beta (64,)
Output:   (32, 32, 8, 16, 32) f32

Math:
  sum_weight cancels exactly (LN shift invariance).
  ln = (x - mu) * rho * gamma + beta,  rho = rsqrt(var + eps)
  8*pooled[q, w'] = S - (ga+go)[w']*M4 + 4*(be+bo)[w'] ; out = Gelu(pooled)

Performance design (v4), based on measured TRN2 rates (all vector-ish engines
run ~1 elem/ns/partition, no fast modes; DMA-transpose is descriptor-bound
and unusable at volume; PE matmuls cost ~30-80 ns each):

  * x is sent twice from the host: normal layout [nc, (k,h,dd,w)] and
    pre-transposed [(dd,w), (k,h,nc)].  The extra 8.4 MB HBM read replaces
    ~100us of on-device xbar transposes.
  * Row sums (sum x, sum x^2 over W per d-parity) run on the otherwise-idle
    PE: per h-block, stationary = xT/sqT block [128, 128nc], moving = the
    2-column dd-parity selector -> psum [nc, h, dd].  Cheap 1-bank PSUM,
    drained by a 64-elem DVE copy.
  * x^2 on ACT (square shares a table set with gelu - no table thrash).
  * DVE keeps only: xr = x*rstd (f32 out), h-pool, gamma stage, w-pair,
    +correction, batched stats math.  GPSIMD takes the f32 d-pool.
  * Correction z = bw - gw*quadsum(mu*rho) precomputed once, batched.

Layout: data-parallel over batch N (4 per core x 8 cores); partitions = 128
(n,c); chunk = one d-pair in (h, dd, w) order.
"""

import numpy as np

import concourse.bacc as bacc
import concourse.bass as bass
import concourse.tile as tile
from concourse import mybir
from concourse.bass_utils import run_bass_kernel_spmd

P = 128
N, C, D, H, W = 32, 32, 16, 32, 64
NCORES = 8
NPER = N // NCORES
EPS = 1e-5
F32 = mybir.dt.float32
BF16 = mybir.dt.bfloat16
MULT = mybir.AluOpType.mult
ADD = mybir.AluOpType.add
SUB = mybir.AluOpType.subtract

NCHUNK = D // 2  # 8
CHUNK = 2 * H * W  # 4096: (h32, dd2, w64)
ROWS = 2 * H  # 64 rows per chunk, (h, dd) order
HH = H // 2  # 16
WW = W // 2  # 32


def _kernel_body(ctx, tc: tile.TileContext, out_ap, xs, xsT, cons):
    nc = tc.nc
    stt = nc.vector.scalar_tensor_tensor
    ACTF = mybir.ActivationFunctionType

    singles = ctx.enter_context(tc.tile_pool(name="singles", bufs=1))
    statp = ctx.enter_context(tc.tile_pool(name="statp", bufs=1))
    xtp = ctx.enter_context(tc.tile_pool(name="xtp", bufs=2))
    sqp = ctx.enter_context(tc.tile_pool(name="sqp", bufs=2))
    xrp = ctx.enter_context(tc.tile_pool(name="xrp", bufs=2))
    workp = ctx.enter_context(tc.tile_pool(name="workp", bufs=2))
    outp = ctx.enter_context(tc.tile_pool(name="outp", bufs=2))
    psR = ctx.enter_context(tc.tile_pool(name="psR", space="PSUM", bufs=2))
    psS = ctx.enter_context(tc.tile_pool(name="psS", space="PSUM", bufs=2))

    # cons rows (bf16 [5, 128]): sel0, sel1, gamma(64), gw(32), bw(32)
    sel_t = singles.tile([P, 2], BF16)
    nc.sync.dma_start(out=sel_t[:, 0:1], in_=cons[0:1, :].rearrange("a b -> b a"))
    nc.sync.dma_start(out=sel_t[:, 1:2], in_=cons[1:2, :].rearrange("a b -> b a"))
    gam_t = singles.tile([P, W], BF16)
    nc.sync.dma_start(out=gam_t[:], in_=cons[2:3, 0:W].to_broadcast((P, W)))
    gw_t = singles.tile([P, WW], BF16)
    nc.sync.dma_start(out=gw_t[:], in_=cons[3:4, 0:WW].to_broadcast((P, WW)))
    bw_t = singles.tile([P, WW], BF16)
    nc.sync.dma_start(out=bw_t[:], in_=cons[4:5, 0:WW].to_broadcast((P, WW)))
    eps_t = singles.tile([P, 1], F32)
    nc.vector.memset(eps_t[:], EPS)
    inv64_t = singles.tile([P, 1], F32)
    nc.vector.memset(inv64_t[:], 1.0 / W)

    x_all = singles.tile([P, NCHUNK, CHUNK], BF16)  # 64KB/partition
    xsf = xs.rearrange("p k f -> p (k f)")
    r1sb = statp.tile([P, NCHUNK, H, 2], F32, tag="r1")  # (k, h, dd)
    r2sb = statp.tile([P, NCHUNK, H, 2], F32, tag="r2")
    rstd = statp.tile([P, NCHUNK * ROWS], F32, tag="rstd")
    z_all = statp.tile([P, NCHUNK * HH, WW], BF16, tag="z_all")
    outf = out_ap.rearrange("p k f -> p k f")

    def phase_a(k):
        # load; PE row-sums of x and x^2; drains on GPSIMD
        nc.sync.dma_start(out=x_all[:, k], in_=xsf[:, k * CHUNK : (k + 1) * CHUNK])
        xT = xtp.tile([P, H, P], BF16, tag="xT")  # [(dd,w), h, nc]
        nc.sync.dma_start(out=xT[:], in_=xsT[:, k])
        sqT = sqp.tile([P, H, P], BF16, tag="sqT")
        nc.scalar.activation(
            sqT[:].rearrange("p h n -> p (h n)"),
            xT[:].rearrange("p h n -> p (h n)"),
            ACTF.Square,
        )
        pr = psR.tile([P, H, 2], F32, tag="pr")
        ps = psS.tile([P, H, 2], F32, tag="ps")
        for t in range(H):
            nc.tensor.matmul(pr[:, t, :], lhsT=xT[:, t, :], rhs=sel_t[:],
                             start=True, stop=True)
        nc.scalar.copy(out=r1sb[:, k], in_=pr[:])
        for t in range(H):
            nc.tensor.matmul(ps[:, t, :], lhsT=sqT[:, t, :], rhs=sel_t[:],
                             start=True, stop=True)
        nc.scalar.copy(out=r2sb[:, k], in_=ps[:])

    HB = 2  # chunks per stats group (pair)

    def phase_b(h):
        # batched stats for chunks [h*HB, (h+1)*HB)
        ck = slice(h * HB, (h + 1) * HB)
        NSH = HB * ROWS  # 128
        r1f = r1sb[:, ck].rearrange("p k h dd -> p (k h dd)")
        r2f = r2sb[:, ck].rearrange("p k h dd -> p (k h dd)")
        rsh = rstd[:, h * NSH : (h + 1) * NSH]
        # var path on GPSIMD: t = r2 - (r1/64)*r1  (= 64*var)
        ra = statp.tile([P, NSH], F32, tag=f"ra{h}")
        nc.gpsimd.tensor_mul(ra[:], r1f, inv64_t[:].to_broadcast((P, NSH)))
        s1sq = statp.tile([P, NSH], F32, tag=f"s1sq{h}")
        nc.gpsimd.tensor_mul(s1sq[:], ra[:], r1f)
        t64 = statp.tile([P, NSH], F32, tag=f"t64{h}")
        nc.gpsimd.tensor_sub(t64[:], r2f, s1sq[:])
        sqv = statp.tile([P, NSH], F32, tag=f"sqv{h}")
        nc.scalar.activation(
            sqv[:], t64[:], ACTF.Sqrt, bias=eps_t[:], scale=1.0 / W
        )
        nc.vector.reciprocal_approx_fast(out=rsh, in_=sqv[:])
        mrs = statp.tile([P, NSH], F32, tag=f"mrs{h}")
        nc.gpsimd.tensor_mul(mrs[:], r1f, rsh)
        mrs4 = mrs[:].rearrange("p (k h dd) -> p k h dd", k=HB, dd=2)
        m1 = statp.tile([P, HB, H], F32, tag=f"m1{h}")
        nc.gpsimd.tensor_add(m1[:], mrs4[:, :, :, 0], mrs4[:, :, :, 1])
        m1p = m1[:].rearrange("p k (hh t) -> p k hh t", t=2)
        mq = statp.tile([P, HB, HH], F32, tag=f"mq{h}")
        nc.gpsimd.tensor_add(mq[:], m1p[:, :, :, 0], m1p[:, :, :, 1])
        NQH = HB * HH  # 32
        mqf = mq[:].rearrange("p k h -> p (k h)")
        zneg = statp.tile([P, NQH, WW], BF16, tag=f"zneg{h}")
        stt(out=zneg[:], in0=mqf.unsqueeze(2).to_broadcast((P, NQH, WW)),
            scalar=-1.0, in1=gw_t[:].unsqueeze(1).to_broadcast((P, NQH, WW)),
            op0=MULT, op1=MULT)
        stt(out=z_all[:, h * NQH : (h + 1) * NQH], in0=zneg[:], scalar=1.0,
            in1=bw_t[:].unsqueeze(1).to_broadcast((P, NQH, WW)),
            op0=MULT, op1=ADD)

    def phase_c(k0):
        # two chunks interleaved per step; all bf16
        ks = range(k0, k0 + 2)
        xrs, xds, xhs, xgs, sps, prs = {}, {}, {}, {}, {}, {}
        for k in ks:
            xc = x_all[:, k].rearrange("p (r w) -> p r w", w=W)
            xr = xrp.tile([P, ROWS, W], BF16, tag=f"xr{k % 2}")
            stt(out=xr[:], in0=xc, scalar=1.0,
                in1=rstd[:, k * ROWS : (k + 1) * ROWS]
                .unsqueeze(2).to_broadcast((P, ROWS, W)),
                op0=MULT, op1=MULT)
            xrs[k] = xr
        for k in ks:
            xr4 = xrs[k][:].rearrange("p (h dd) w -> p h dd w", dd=2)
            xd = workp.tile([P, H, W], BF16, tag=f"xd{k % 2}")
            stt(out=xd[:], in0=xr4[:, :, 0, :], scalar=1.0,
                in1=xr4[:, :, 1, :], op0=MULT, op1=ADD)
            xds[k] = xd
        for k in ks:
            xd4 = xds[k][:].rearrange("p (hh t) w -> p hh t w", t=2)
            xh = workp.tile([P, HH, W], BF16, tag=f"xh{k % 2}")
            stt(out=xh[:], in0=xd4[:, :, 0, :], scalar=1.0,
                in1=xd4[:, :, 1, :], op0=MULT, op1=ADD)
            xhs[k] = xh
        for k in ks:
            xg = workp.tile([P, HH, W], BF16, tag=f"xg{k % 2}")
            stt(out=xg[:], in0=xhs[k][:], scalar=1.0,
                in1=gam_t[:].unsqueeze(1).to_broadcast((P, HH, W)),
                op0=MULT, op1=MULT)
            xgs[k] = xg
        for k in ks:
            xg4 = xgs[k][:].rearrange("p h (v t) -> p h v t", t=2)
            spre = workp.tile([P, HH, WW], BF16, tag=f"sp{k % 2}")
            stt(out=spre[:], in0=xg4[:, :, :, 0], scalar=1.0,
                in1=xg4[:, :, :, 1], op0=MULT, op1=ADD)
            sps[k] = spre
        for k in ks:
            pre = workp.tile([P, HH, WW], BF16, tag=f"pr{k % 2}")
            stt(out=pre[:], in0=sps[k][:], scalar=1.0,
                in1=z_all[:, k * HH : (k + 1) * HH], op0=MULT, op1=ADD)
            prs[k] = pre
        for k in ks:
            res = outp.tile([P, HH * WW], BF16, tag=f"res{k % 2}")
            nc.scalar.activation(
                res[:], prs[k][:].rearrange("p a b -> p (a b)"),
                ACTF.Gelu, scale=0.125,
            )
            nc.sync.dma_start(out=outf[:, k], in_=res[:])

    # emission: A(first half) -> B0 -> A(second half) || C(first half) -> B1
    # -> C(second half).  A's drains ride GPSIMD so C owns the DVE queue.
    phase_a(0); phase_a(1); phase_b(0)
    phase_a(2); phase_a(3); phase_b(1)
    phase_c(0)
    phase_a(4); phase_a(5); phase_b(2)
    phase_c(2)
    phase_a(6); phase_a(7); phase_b(3)
    phase_c(4)
    phase_c(6)


_CACHE: dict = {}


def _get_compiled():
    if "nc" not in _CACHE:
        nc = bacc.Bacc("TRN2", target_bir_lowering=False, debug=False)
        xs = nc.dram_tensor("xs", [P, NCHUNK, CHUNK], BF16, kind="ExternalInput").ap()
        xsT = nc.dram_tensor(
            "xsT", [P, NCHUNK, H, P], BF16, kind="ExternalInput"
        ).ap()
        cons = nc.dram_tensor("cons", [5, P], BF16, kind="ExternalInput").ap()
        out = nc.dram_tensor(
            "out", [P, NCHUNK, HH * WW], BF16, kind="ExternalOutput"
        ).ap()
        from contextlib import ExitStack

        with tile.TileContext(nc) as tc, ExitStack() as ctx:
            _kernel_body(ctx, tc, out, xs, xsT, cons)
        nc.compile()
        _CACHE["nc"] = nc
    return _CACHE["nc"]


def _make_consts(gamma: np.ndarray, beta: np.ndarray):
    import ml_dtypes

    cons = np.zeros((5, P), np.float32)
    dd_of_p = (np.arange(P) // W).astype(np.int32)
    cons[0] = (dd_of_p == 0).astype(np.float32)
    cons[1] = (dd_of_p == 1).astype(np.float32)
    cons[2, 0:W] = gamma
    cons[3, 0:WW] = (gamma[0::2] + gamma[1::2]) / float(W)  # mrs carries 64x
    cons[4, 0:WW] = 4.0 * (beta[0::2] + beta[1::2])
    return cons.astype(ml_dtypes.bfloat16)


def kernel(x, sum_weight, gamma, beta, trace=False):
    import ml_dtypes

    del sum_weight  # cancels exactly in LayerNorm (shift invariance)
    nc = _get_compiled()
    x = np.asarray(x)
    if x.dtype != ml_dtypes.bfloat16:
        x = x.astype(ml_dtypes.bfloat16)
    # (N,C,D,H,W) -> (N,C,k,h,dd,w)
    xp = np.ascontiguousarray(
        x.reshape(N, C, NCHUNK, 2, H, W).transpose(0, 1, 2, 4, 3, 5)
    )
    cons = _make_consts(
        np.asarray(gamma, dtype=np.float32), np.asarray(beta, dtype=np.float32)
    )
    in_maps = []
    for core in range(NCORES):
        shard = xp[core * NPER : (core + 1) * NPER].reshape(P, NCHUNK, H, 2, W)
        # transposed copy: [(dd,w), k, h, nc]
        shT = np.ascontiguousarray(shard.transpose(3, 4, 1, 2, 0)).reshape(
            P, NCHUNK, H, P
        )
        in_maps.append(
            {
                "xs": np.ascontiguousarray(shard.reshape(P, NCHUNK, CHUNK)),
                "xsT": shT,
                "cons": cons,
            }
        )
    res = run_bass_kernel_spmd(nc, in_maps, core_ids=list(range(NCORES)), trace=trace)
    out = np.concatenate(
        [
            res.results[i]["out"]
            .astype(np.float32)
            .reshape(NPER, C, NCHUNK, HH, WW)
            for i in range(NCORES)
        ],
        axis=0,
    )
    if trace:
        return out, res
    return out


if __name__ == "__main__":
    rng = np.random.default_rng(0)
    x = rng.standard_normal((N, C, D, H, W), dtype=np.float32)
    sw = rng.standard_normal((1,)).astype(np.float32)
    gamma = rng.random((W,), dtype=np.float32)
    beta = rng.standard_normal((W,)).astype(np.float32)
    y = kernel(x, sw, gamma, beta)
    print(y.shape, y.dtype)
